# revision 13
# baseline (speedup 1.0000x reference)
"""DiT block (Linformer attention + adaLN + MLP) on 8 TRN2 NeuronCores.

Sharding: data-parallel over batch (B=8 -> one batch element per core).

v2 layout (per core, S=2048 tokens, D=1024 features):
 - Phase A conditioning matvecs run in f32r (single-pass fp32 at full PE
   rate for moving dims >= 256); scale/offset rows are broadcast to 128
   partitions with gpsimd.partition_broadcast instead of PE ones-matmuls.
 - All other weights are converted once to f16 via DRAM->DRAM casting DMAs
   on the gpsimd (SWDGE) queue, then streamed as plain f16 HWDGE loads:
   no per-use casts, and the MLP weights are no longer re-streamed per
   token group (m1w resident in SBUF, m2w streamed f16 once per group).
 - Attention: per half-group (8 heads) all scores matmuls are issued as one
   64-row-tiled block, exps on scalar, then all attn@V matmuls; softmax
   denominators come from a fused ones-column appended to v_proj, and the
   1/denom normalization uses gpsimd partition_broadcast + vector
   reciprocal + psum multiply (no PE broadcast matmuls).
 - wo+residual is fused with adaLN2 + transposes per 128-token chunk, so
   attn_out is written to DRAM once and only re-read for the final
   residual add.

DMA queues: sync carries x/attn_sc/out streams and the f16 attention
weights; scalar carries phase-A f32r weights, EF/w1/w2 f16 streams and
bias rows; gpsimd (SWDGE) carries the 8 one-time casts plus all
latency-chained tiny transfers (h->cols, partition broadcasts) so they
never block a bulk queue at its head.
"""
import contextlib

import numpy as np

import concourse.bass as bass
import concourse.mybir as mybir
import concourse.tile as tile
from concourse import bacc
from concourse.bass import ds, ts
from concourse.bass_utils import run_bass_kernel_spmd
from concourse.masks import make_identity

f32 = mybir.dt.float32
f32r = mybir.dt.float32r
f16 = mybir.dt.float16
f8 = mybir.dt.float8e4
AF = mybir.ActivationFunctionType
OP = mybir.AluOpType

B, S, D, H, K, MLP, ZD = 8, 2048, 1024, 16, 256, 4096, 1024
DH = D // H      # 64
P = 128
SC = S // P      # 16 token chunks of 128
DC = D // P      # 8 feature chunks of 128
NG = 4           # token groups of 512
GS = 512
MC = MLP // P    # 32
KC = K // P      # 2
EPS = 1e-6

W2D = [("wq", D, D), ("wk", D, D), ("wv", D, D), ("wo", D, D),
       ("Ew", S, K), ("Fw", S, K),
       ("m1w", D, MLP), ("m2w", MLP, D)]
W2DR = [("h1w", ZD, D), ("g1w", D, D), ("be1w", D, D),
        ("h2w", ZD, D), ("g2w", D, D), ("be2w", D, D)]
W1D = [("bq", D), ("bk", D), ("bv", D), ("bo", D), ("Eb", K), ("Fb", K),
       ("h1b", D), ("g1b", D), ("be1b", D), ("h2b", D), ("g2b", D), ("be2b", D),
       ("m1b", MLP), ("m2b", D)]

_cache = {}


def build():
    if "nc" in _cache:
        return _cache["nc"]
    nc = bacc.Bacc("TRN2", target_bir_lowering=False, debug=False, num_devices=8)
    ap = {}
    ap["x"] = nc.dram_tensor("x", [S, D], f32, kind="ExternalInput").ap()
    ap["z"] = nc.dram_tensor("z", [1, ZD], f32r, kind="ExternalInput").ap()
    for nm, a, b in W2D:
        ap[nm] = nc.dram_tensor(nm, [a, b], f32, kind="ExternalInput").ap()
    for nm, a, b in W2DR:
        ap[nm] = nc.dram_tensor(nm, [a, b], f32r, kind="ExternalInput").ap()
    for nm, a in W1D:
        ap[nm] = nc.dram_tensor(nm, [a], f32, kind="ExternalInput").ap()
    out = nc.dram_tensor("out", [S, D], f32, kind="ExternalOutput").ap()
    with tile.TileContext(nc, trace_sim=False) as tc:
        _emit(nc, tc, ap, out)
    nc.compile()
    _cache["nc"] = nc
    return nc


def _emit(nc, tc, ap, out):
    ctx = contextlib.ExitStack()
    with ctx:
        # ---------- whole-kernel pools ----------
        const = ctx.enter_context(tc.tile_pool(name="const", bufs=1))
        cols = ctx.enter_context(tc.tile_pool(name="cols", bufs=1))
        bc = ctx.enter_context(tc.tile_pool(name="bc", bufs=1))
        dram = ctx.enter_context(tc.tile_pool(name="dram", bufs=1, space="DRAM"))

        attn_sc = [dram.tile([P, D], f32, tag=f"attn_sc{i}", name=f"attn_sc{i}")
                   for i in range(SC)]

        ident_f = const.tile([P, P], f32, tag="ident_f", name="ident_f")
        make_identity(nc, ident_f)
        ident_h = const.tile([P, P], f16, tag="ident_h", name="ident_h")
        nc.vector.tensor_copy(ident_h[:], ident_f[:])
        eps_t = const.tile([P, 1], f32, tag="eps", name="eps")
        nc.vector.memset(eps_t[:], EPS)
        ones_f = const.tile([P, 1], f32, tag="ones_f", name="ones_f")
        nc.vector.memset(ones_f[:], 1.0)
        onescol_h = const.tile([P, 1], f16, tag="onescol_h", name="onescol_h")
        nc.vector.tensor_copy(onescol_h[:], ones_f[:])

        def col_load(name, n):
            """1-D DRAM vector [n*128] -> sbuf [128, n] (partition-major)."""
            t = cols.tile([P, n], f32, tag=f"cols_{name}", name=f"cols_{name}")
            for j in range(n):
                nc.scalar.dma_start(t[:, j:j + 1], ap[name][ds(P * j, P)])
            return t

        # broadcast result tiles (f16, whole-kernel)
        scale1_b = bc.tile([P, D], f16, tag="scale1_b", name="scale1_b")
        offset1_b = bc.tile([P, D], f16, tag="offset1_b", name="offset1_b")
        scale2_b = bc.tile([P, D], f16, tag="scale2_b", name="scale2_b")
        offset2_b = bc.tile([P, D], f16, tag="offset2_b", name="offset2_b")
        bo_b = bc.tile([P, D], f16, tag="bo_b", name="bo_b")
        m2b_b = bc.tile([P, D], f16, tag="m2b_b", name="m2b_b")

        zc_f = cols.tile([P, DC], f32r, tag="zc_f", name="zc_f")
        for j in range(DC):
            nc.scalar.dma_start(zc_f[:, j:j + 1], ap["z"][0:1, ds(P * j, P)])

        def vec_layer(vsb, vps, rowp, wname, lhs_cols, bias_row, act, out_row):
            """out_row[1, D] = act(lhs^T @ w + bias) with f32r weights."""
            pts = [vps.tile([1, GS], f32, tag=f"vps{h}", name=f"vps{h}")
                   for h in range(2)]
            for j in range(DC):
                wt = vsb.tile([P, D], f32r, tag=f"vw_{wname}", name=f"vw_{wname}",
                              bufs=4)
                nc.scalar.dma_start(wt[:], ap[wname][ds(P * j, P), :])
                for h in range(2):
                    nc.tensor.matmul(pts[h][:], lhs_cols[:, j:j + 1],
                                     wt[:, ds(GS * h, GS)],
                                     start=(j == 0), stop=(j == DC - 1))
            for h in range(2):
                pre = rowp.tile([1, GS], f32, tag=f"vpre{h}", name=f"vpre{h}",
                                bufs=2)
                nc.vector.tensor_add(pre[:], pts[h][:],
                                     bias_row[0:1, ds(GS * h, GS)])
                if act is None:
                    nc.vector.tensor_copy(out_row[0:1, ds(GS * h, GS)], pre[:])
                else:
                    nc.scalar.activation(out_row[0:1, ds(GS * h, GS)],
                                         pre[:], act)

        def a_chain(vsb, vps, rowp, hw, hb, gw, gb, bw, bb, tagn, sc_b, of_b):
            def row_load(name):
                t = rowp.tile([1, D], f32, tag="arow_b", name=f"row_{name}",
                              bufs=2)
                nc.scalar.dma_start(t[:], ap[name][0:D])
                return t

            hb_row = row_load(hb)
            h_row = rowp.tile([1, D], f32, tag="h_row", name=f"h_{tagn}")
            vec_layer(vsb, vps, rowp, hw, zc_f, hb_row, AF.Silu, h_row)
            h_row_r = rowp.tile([1, D], f32r, tag="h_row_r", name=f"hr_{tagn}")
            nc.vector.tensor_copy(h_row_r[:], h_row[:])
            h_c = cols.tile([P, DC], f32r, tag=f"c_{tagn}", name=f"c_{tagn}")
            for j in range(DC):
                nc.gpsimd.dma_start(h_c[:, j:j + 1], h_row_r[0:1, ds(P * j, P)])
            gb_row = row_load(gb)
            sc_row = rowp.tile([1, D], f32, tag="sc_row", name=f"sc_{tagn}")
            vec_layer(vsb, vps, rowp, gw, h_c, gb_row, None, sc_row)
            sc_row_h = rowp.tile([1, D], f16, tag="sc_row_h", name=f"sch_{tagn}")
            nc.vector.tensor_copy(sc_row_h[:], sc_row[:])
            nc.gpsimd.partition_broadcast(sc_b[:], sc_row_h[0:1, :])
            bb_row = row_load(bb)
            of_row = rowp.tile([1, D], f32, tag="of_row", name=f"of_{tagn}")
            vec_layer(vsb, vps, rowp, bw, h_c, bb_row, None, of_row)
            of_row_h = rowp.tile([1, D], f16, tag="of_row_h", name=f"ofh_{tagn}")
            nc.vector.tensor_copy(of_row_h[:], of_row[:])
            nc.gpsimd.partition_broadcast(of_b[:], of_row_h[0:1, :])

        # =========== phase A1: adaLN1 conditioning vectors (f32r) ===========
        with (
            tc.tile_pool(name="vec1_sb", bufs=1) as vsb1,
            tc.tile_pool(name="row1_sb", bufs=1) as rowp1,
            tc.tile_pool(name="vec1_ps", bufs=2, space="PSUM") as vps1,
        ):
            a_chain(vsb1, vps1, rowp1, "h1w", "h1b", "g1w", "g1b",
                    "be1w", "be1b", "h1", scale1_b, offset1_b)
        # manual pool stacks (LIFO per SBUF side)
        s_woh = contextlib.ExitStack()   # left (created first: popped last)
        s_qT = contextlib.ExitStack()    # left
        s_x1n = contextlib.ExitStack()   # left
        s_x2T = contextlib.ExitStack()   # left
        s_w1 = contextlib.ExitStack()    # left
        s_w2 = contextlib.ExitStack()    # left (E only)
        s_kv = contextlib.ExitStack()    # right (kpT/vpe, lingers under aoT)
        s_pef = contextlib.ExitStack()   # right (pefE/F + colsums, popped at KV end)
        s_aoT = contextlib.ExitStack()   # right
        s_hm = contextlib.ExitStack()    # right
        try:
            # ===== phase B: adaLN1 + transposes + qT =====
            qT_p = s_qT.enter_context(tc.tile_pool(name="qT", bufs=1))
            qT = [[qT_p.tile([P, GS], f16, tag=f"qT_{j}_{g}", name=f"qT_{j}_{g}")
                   for g in range(NG)] for j in range(DC)]
            x1n_p = s_x1n.enter_context(tc.tile_pool(name="x1nat", bufs=1))
            x1n = []
            with (
                tc.tile_pool(name="wq_sb", bufs=1) as wqsb,
                tc.tile_pool(name="ln1_sb", bufs=2) as ln_sb,
                tc.tile_pool(name="x1Trot", bufs=1) as x1t_p,
                tc.tile_pool(name="tp1_ps", bufs=2, space="PSUM") as ln_ps,
                tc.tile_pool(name="q_ps", bufs=3, space="PSUM") as qps,
            ):
                wq_r = []
                for j in range(DC):
                    wf = wqsb.tile([P, D], f32, tag="wq_f", name="wq_f",
                                   bufs=2)
                    nc.scalar.dma_start(wf[:], ap["wq"][ds(P * j, P), :])
                    wr = wqsb.tile([P, D], f16, tag=f"wq_r{j}", name=f"wq_r{j}")
                    nc.scalar.copy(wr[:], wf[:])
                    wq_r.append(wr)
                bq_c = col_load("bq", DC)
                bk_c = col_load("bk", DC)
                Fb_c = col_load("Fb", KC)
                for g in range(NG):
                    x1T_g = [x1t_p.tile([P, GS], f16, tag=f"x1T_{j}",
                                        name=f"x1T_{j}") for j in range(DC)]
                    for ii in range(4):
                        i = 4 * g + ii
                        xt = ln_sb.tile([P, D], f32, tag="ln_in",
                                        name=f"ln_in{i}", bufs=4)
                        nc.sync.dma_start(xt[:], ap["x"][ds(P * i, P), :])
                        st = ln_sb.tile([P, 2, 6], f32, tag="ln_st",
                                        name="ln_st")
                        nc.vector.bn_stats(st[:, 0, :], xt[:, 0:GS])
                        nc.vector.bn_stats(st[:, 1, :], xt[:, GS:D])
                        mv = ln_sb.tile([P, 2], f32, tag="ln_mv", name="ln_mv")
                        nc.vector.bn_aggr(mv[:], st[:])
                        sd = ln_sb.tile([P, 1], f32, tag="ln_sd",
                                        name="ln_sd")
                        nc.scalar.activation(sd[:], mv[:, 1:2], AF.Sqrt,
                                             bias=eps_t[:])
                        rstd = ln_sb.tile([P, 1], f32, tag="ln_rstd",
                                          name="ln_rstd")
                        nc.vector.reciprocal_approx_fast(rstd[:], sd[:])
                        nmr = ln_sb.tile([P, 1], f32, tag="ln_nmr",
                                         name="ln_nmr")
                        nc.vector.tensor_scalar(nmr[:], mv[:, 0:1], rstd[:],
                                                -1.0, OP.mult, OP.mult)
                        xn = ln_sb.tile([P, D], f32, tag="ln_xn", name="ln_xn")
                        nc.scalar.activation(xn[:], xt[:], AF.Identity,
                                             bias=nmr[:], scale=rstd[:])
                        nc.gpsimd.tensor_mul(xn[:], xn[:], scale1_b[:])
                        x1t = x1n_p.tile([P, D], f16, tag=f"nat{i}",
                                         name=f"nat{i}")
                        nc.vector.tensor_add(x1t[:], xn[:], offset1_b[:])
                        x1n.append(x1t)
                        for j in range(DC):
                            pt = ln_ps.tile([P, P], f16, tag="tp_ps",
                                            name="tp_ps")
                            nc.tensor.transpose(pt[:], x1t[:, ds(P * j, P)],
                                                ident_h[:])
                            nc.vector.tensor_copy(
                                x1T_g[j][:, ds(P * ii, P)], pt[:])
                    for jo in range(DC):
                        pt = qps.tile([P, GS], f32, tag="q_ps", name="q_ps")
                        for j in range(DC):
                            nc.tensor.matmul(pt[:],
                                             wq_r[j][:, ds(P * jo, P)],
                                             x1T_g[j][:],
                                             start=(j == 0),
                                             stop=(j == DC - 1))
                        nc.scalar.activation(qT[jo][g][:], pt[:], AF.Identity,
                                             bias=bq_c[:, jo:jo + 1])

            # ===== phase B2: P_EF = x1^T @ [Ew|Fw], colsums fused after =====
            kv_sb = s_kv.enter_context(
                tc.tile_pool(name="kv_sb", bufs=1, side="right"))
            pef_sb = s_pef.enter_context(
                tc.tile_pool(name="pef_sb", bufs=1, side="right"))
            pefE = [pef_sb.tile([P, K], f16, tag=f"pefE{j}", name=f"pefE{j}")
                    for j in range(DC)]
            pefF = [pef_sb.tile([P, K], f16, tag=f"pefF{j}", name=f"pefF{j}")
                    for j in range(DC)]
            cs_row = pef_sb.tile([1, 2 * K], f32, tag="cs", name="cs")
            with tc.tile_pool(name="ef_sb", bufs=1) as efsb:
                ef_h = []
                for i in range(SC):
                    ff = efsb.tile([P, 2 * K], f32, tag="ef_f", name="ef_f",
                                   bufs=3)
                    nc.scalar.dma_start(ff[:, 0:K], ap["Ew"][ds(P * i, P), :])
                    nc.scalar.dma_start(ff[:, K:2 * K],
                                        ap["Fw"][ds(P * i, P), :])
                    t = efsb.tile([P, 2 * K], f16, tag=f"ef{i}", name=f"ef{i}")
                    nc.scalar.copy(t[:], ff[:])
                    ef_h.append(t)
                m1b_c = col_load("m1b", MC)
                with tc.tile_pool(name="pef_ps", bufs=1, space="PSUM") as pfps:
                    pef_ps = [pfps.tile([P, 2 * K], f32, tag=f"pefp{j}",
                                        name=f"pefp{j}") for j in range(DC)]
                    for i in range(SC):
                        for j in range(DC):
                            nc.tensor.matmul(pef_ps[j][:],
                                             x1n[i][:, ds(P * j, P)],
                                             ef_h[i][:],
                                             start=(i == 0), stop=(i == SC - 1))
                    for j in range(DC):
                        nc.scalar.copy(pefE[j][:], pef_ps[j][:, 0:K])
                        nc.scalar.copy(pefF[j][:], pef_ps[j][:, K:2 * K])
                with tc.tile_pool(name="cs_ps", bufs=1, space="PSUM") as csps:
                    cs_ps = csps.tile([1, 2 * K], f32, tag="cs_ps",
                                      name="cs_ps")
                    for i in range(SC):
                        nc.tensor.matmul(cs_ps[:], onescol_h[:], ef_h[i][:],
                                         start=(i == 0), stop=(i == SC - 1))
                    nc.vector.tensor_copy(cs_row[:], cs_ps[:])
            s_x1n.close()

            # ===== phase KV: k_projT, v_proj_ext =====
            kpT = [kv_sb.tile([P, K], f16, tag=f"kpT{j}", name=f"kpT{j}")
                   for j in range(DC)]
            vpe = [kv_sb.tile([P, 65 * H], f16, tag=f"vpe{c}", name=f"vpe{c}")
                   for c in range(KC)]
            with (
                tc.tile_pool(name="kv_w", bufs=1) as kvw,
                tc.tile_pool(name="kv_bias", bufs=1) as kvb,
                tc.tile_pool(name="kv_tmp", bufs=2) as kvt,
                tc.tile_pool(name="kv_ps", bufs=2, space="PSUM") as kvps,
            ):
                Eb_row = kvb.tile([1, K], f32, tag="Eb_row", name="Eb_row")
                nc.scalar.dma_start(Eb_row[:], ap["Eb"][0:K])
                Eb_b = kvb.tile([P, K], f32, tag="Eb_b", name="Eb_b")
                nc.gpsimd.partition_broadcast(Eb_b[:], Eb_row[0:1, :])
                csE_b = kvb.tile([P, K], f32, tag="csE_b", name="csE_b")
                nc.gpsimd.partition_broadcast(csE_b[:], cs_row[0:1, 0:K])
                bv_row = kvb.tile([1, D], f32, tag="bv_row", name="bv_row")
                nc.scalar.dma_start(bv_row[:], ap["bv"][0:D])
                bv_b = kvb.tile([P, D], f32, tag="bv_b", name="bv_b")
                nc.gpsimd.partition_broadcast(bv_b[:], bv_row[0:1, :])
                csF_c = kvb.tile([P, KC], f32, tag="csF_c", name="csF_c")
                for c in range(KC):
                    nc.gpsimd.dma_start(csF_c[:, c:c + 1],
                                        cs_row[0:1, ds(K + P * c, P)])
                kp_bias = []
                for j in range(DC):
                    bt = kvb.tile([P, K], f32, tag=f"kpb{j}", name=f"kpb{j}")
                    nc.vector.tensor_scalar(bt[:], csE_b[:], bk_c[:, j:j + 1],
                                            None, OP.mult)
                    nc.vector.tensor_add(bt[:], bt[:], Eb_b[:])
                    kp_bias.append(bt)
                vp_bias = []
                for c in range(KC):
                    bt = kvb.tile([P, D], f32, tag=f"vpb{c}", name=f"vpb{c}")
                    nc.vector.tensor_scalar(bt[:], bv_b[:], csF_c[:, c:c + 1],
                                            Fb_c[:, c:c + 1], OP.mult, OP.add)
                    vp_bias.append(bt)

                wk_r, wv_r = [], []
                for j in range(DC):
                    for nm, lst, tg in (("wk", wk_r, "wk"), ("wv", wv_r, "wv")):
                        wf = kvw.tile([P, D], f32, tag=f"{tg}_f",
                                      name=f"{tg}_f", bufs=2)
                        nc.sync.dma_start(wf[:], ap[nm][ds(P * j, P), :])
                        wr = kvw.tile([P, D], f16, tag=f"{tg}_r{j}",
                                      name=f"{tg}_r{j}")
                        nc.scalar.copy(wr[:], wf[:])
                        lst.append(wr)
                for jo in range(DC):
                    pt = kvps.tile([P, K], f32, tag="kp_ps", name="kp_ps")
                    for j in range(DC):
                        nc.tensor.matmul(pt[:], wk_r[j][:, ds(P * jo, P)],
                                         pefE[j][:],
                                         start=(j == 0), stop=(j == DC - 1))
                    nc.vector.tensor_add(kpT[jo][:], pt[:], kp_bias[jo][:])
                for hf in range(2):
                    for c in range(KC):
                        pt = kvps.tile([P, GS], f32, tag="vp_ps", name="vp_ps")
                        for j in range(DC):
                            nc.tensor.matmul(pt[:], pefF[j][:, ds(P * c, P)],
                                             wv_r[j][:, ds(GS * hf, GS)],
                                             start=(j == 0), stop=(j == DC - 1))
                        tmp = kvt.tile([P, GS], f32, tag="vp_tmp",
                                       name="vp_tmp")
                        nc.vector.tensor_add(tmp[:], pt[:],
                                             vp_bias[c][:, ds(GS * hf, GS)])
                        for hh in range(8):
                            h = 8 * hf + hh
                            nc.vector.tensor_copy(vpe[c][:, ds(65 * h, 64)],
                                                  tmp[:, ds(64 * hh, 64)])
                for c in range(KC):
                    for h in range(H):
                        nc.vector.tensor_copy(vpe[c][:, ds(65 * h + 64, 1)],
                                              ones_f[:, 0:1])

            s_pef.close()

            # ===== phase A2: adaLN2 conditioning vectors =====
            with (
                tc.tile_pool(name="vec2_sb", bufs=1) as vsb2,
                tc.tile_pool(name="row2_sb", bufs=1) as rowp2,
                tc.tile_pool(name="vec2_ps", bufs=2, space="PSUM") as vps2,
            ):
                a_chain(vsb2, vps2, rowp2, "h2w", "h2b", "g2w", "g2b",
                        "be2w", "be2b", "h2", scale2_b, offset2_b)
                brow = rowp2.tile([1, D], f32, tag="brow", name="bo_row",
                                  bufs=2)
                nc.scalar.dma_start(brow[:], ap["bo"][0:D])
                browh = rowp2.tile([1, D], f16, tag="browh", name="bo_row_h",
                                   bufs=2)
                nc.vector.tensor_copy(browh[:], brow[:])
                nc.gpsimd.partition_broadcast(bo_b[:], browh[0:1, :])
                brow2 = rowp2.tile([1, D], f32, tag="brow", name="m2b_row",
                                   bufs=2)
                nc.scalar.dma_start(brow2[:], ap["m2b"][0:D])
                brow2h = rowp2.tile([1, D], f16, tag="browh", name="m2b_row_h",
                                    bufs=2)
                nc.vector.tensor_copy(brow2h[:], brow2[:])
                nc.gpsimd.partition_broadcast(m2b_b[:], brow2h[0:1, :])

            # ===== load wo (f16) for C3 =====
            woh_sb = s_woh.enter_context(
                tc.tile_pool(name="woh_sb", bufs=1, side="right"))
            wo_r = []
            for j in range(DC):
                wf = woh_sb.tile([P, D], f32, tag="wo_f", name="wo_f", bufs=2)
                nc.sync.dma_start(wf[:], ap["wo"][ds(P * j, P), :])
                wr = woh_sb.tile([P, D], f16, tag=f"wo_r{j}", name=f"wo_r{j}")
                nc.vector.tensor_copy(wr[:], wf[:])
                wo_r.append(wr)

            # ===== phase C2: attention =====
            aoT_p = s_aoT.enter_context(
                tc.tile_pool(name="aoT", bufs=1, side="right"))
            aoT = [[aoT_p.tile([P, GS], f16, tag=f"aoT_{j}_{g}",
                               name=f"aoT_{j}_{g}")
                    for g in range(NG)] for j in range(DC)]
            with (
                tc.tile_pool(name="exp_sb", bufs=1) as expsb,
                tc.tile_pool(name="nrm_sb", bufs=1) as nrmsb,
                tc.tile_pool(name="sc_ps", bufs=4, space="PSUM") as scps,
                tc.tile_pool(name="av_ps", bufs=3, space="PSUM") as avps,
            ):
                def emit_scores_half(hg):
                    """scores+exp for the 8 heads of half-group hg (0..7)."""
                    g, hb = hg // 2, (hg % 2) * 8
                    exps = []
                    for ph in range(4):
                        for e in range(2):
                            h = hb + 2 * ph + e
                            j, r0 = h // 2, 64 * (h % 2)
                            ets = []
                            for c in range(KC):
                                spt = scps.tile([P, GS], f32, tag="sc",
                                                name="sc")
                                nc.tensor.matmul(
                                    spt[:],
                                    kpT[j][r0:r0 + 64, ds(P * c, P)],
                                    qT[j][g][r0:r0 + 64, :],
                                    start=True, stop=True)
                                et = expsb.tile([P, GS], f16, tag="exp",
                                                name="exp", bufs=34)
                                nc.scalar.activation(et[:], spt[:], AF.Exp,
                                                     scale=0.125)
                                ets.append(et)
                            exps.append((h, ets))
                    return exps

                def emit_av_half(exps, g):
                    for h, ets in exps:
                        j, r0 = h // 2, 64 * (h % 2)
                        apt = avps.tile([65, GS], f32, tag="av", name="av")
                        for c in range(KC):
                            nc.tensor.matmul(apt[:], vpe[c][:, ds(65 * h, 65)],
                                             ets[c][:],
                                             start=(c == 0), stop=(c == KC - 1))
                        den = nrmsb.tile([1, GS], f32, tag="den", name="den",
                                         bufs=4)
                        nc.vector.tensor_copy(den[:], apt[64:65, :])
                        den_b = nrmsb.tile([64, GS], f32, tag="den_b",
                                           name="den_b", bufs=3)
                        nc.gpsimd.partition_broadcast(den_b[:], den[0:1, :])
                        rec = nrmsb.tile([64, GS], f32, tag="rec", name="rec",
                                         bufs=3)
                        nc.vector.reciprocal_approx_fast(rec[:], den_b[:])
                        nc.vector.tensor_mul(aoT[j][g][r0:r0 + 64, :],
                                             apt[0:64, :], rec[:])

                prev = None
                for hg in range(2 * NG):
                    cur = emit_scores_half(hg)
                    if prev is not None:
                        emit_av_half(*prev)
                    prev = (cur, hg // 2)
                emit_av_half(*prev)
            s_qT.close()

            # ===== phase C3: wo + residual + adaLN2 + transposes, fused =====
            x2T_p = s_x2T.enter_context(tc.tile_pool(name="x2T", bufs=1))
            x2T = [x2T_p.tile([P, S], f16, tag=f"x2T_{j}", name=f"x2T_{j}")
                   for j in range(DC)]
            # w1 resident as fp8 (x16 scaled; folded back in the gelu),
            # streamed f32 + cast on gpsimd during C3
            w1sb_p = s_w1.enter_context(tc.tile_pool(name="w1sb", bufs=1))
            w1sb = []
            for j in range(DC):
                wf = w1sb_p.tile([P, MLP], f32, tag="w1_f", name="w1_f",
                                 bufs=2)
                nc.sync.dma_start(wf[:], ap["m1w"][ds(P * j, P), :])
                t = w1sb_p.tile([P, MLP], f8, tag=f"w1_{j}", name=f"w1_{j}")
                nc.gpsimd.tensor_scalar(t[:], wf[:], 16.0, None, OP.mult)
                w1sb.append(t)
            with (
                tc.tile_pool(name="c3_sb", bufs=1) as c3sb,
                tc.tile_pool(name="wo_ps", bufs=3, space="PSUM") as wops,
                tc.tile_pool(name="tp2_ps", bufs=2, space="PSUM") as tp2ps,
            ):
                for i in range(SC):
                    g, c = i // 4, (i % 4) * P
                    xt = c3sb.tile([P, D], f32, tag="res_x", name="res_x",
                                   bufs=2)
                    nc.scalar.dma_start(xt[:], ap["x"][ds(P * i, P), :])
                    at = c3sb.tile([P, D], f32, tag="attn_nat",
                                   name="attn_nat", bufs=2)
                    for hf in range(2):
                        pt = wops.tile([P, GS], f32, tag="wo_ps", name="wo_ps")
                        for j in range(DC):
                            nc.tensor.matmul(pt[:], aoT[j][g][:, ds(c, P)],
                                             wo_r[j][:, ds(GS * hf, GS)],
                                             start=(j == 0), stop=(j == DC - 1))
                        tm = c3sb.tile([P, GS], f32, tag="wo_tmp",
                                       name="wo_tmp", bufs=2)
                        nc.vector.tensor_add(tm[:], pt[:],
                                             bo_b[:, ds(GS * hf, GS)])
                        nc.vector.tensor_add(at[:, ds(GS * hf, GS)], tm[:],
                                             xt[:, ds(GS * hf, GS)])
                    nc.sync.dma_start(attn_sc[i][:], at[:])
                    st = c3sb.tile([P, 2, 6], f32, tag="ln2_st",
                                   name="ln2_st", bufs=2)
                    nc.vector.bn_stats(st[:, 0, :], at[:, 0:GS])
                    nc.vector.bn_stats(st[:, 1, :], at[:, GS:D])
                    mv = c3sb.tile([P, 2], f32, tag="ln2_mv", name="ln2_mv")
                    nc.vector.bn_aggr(mv[:], st[:])
                    sd = c3sb.tile([P, 1], f32, tag="ln2_sd",
                                   name="ln2_sd")
                    nc.scalar.activation(sd[:], mv[:, 1:2], AF.Sqrt,
                                         bias=eps_t[:])
                    rstd = c3sb.tile([P, 1], f32, tag="ln2_rstd",
                                     name="ln2_rstd")
                    nc.vector.reciprocal_approx_fast(rstd[:], sd[:])
                    nmr = c3sb.tile([P, 1], f32, tag="ln2_nmr", name="ln2_nmr")
                    nc.vector.tensor_scalar(nmr[:], mv[:, 0:1], rstd[:],
                                            -1.0, OP.mult, OP.mult)
                    xn = c3sb.tile([P, D], f32, tag="ln2_xn", name="ln2_xn")
                    nc.scalar.activation(xn[:], at[:], AF.Identity,
                                         bias=nmr[:], scale=rstd[:])
                    nc.gpsimd.tensor_mul(xn[:], xn[:], scale2_b[:])
                    x2t = c3sb.tile([P, D], f16, tag="x2nat", name="x2nat",
                                    bufs=2)
                    nc.vector.tensor_add(x2t[:], xn[:], offset2_b[:])
                    for j in range(DC):
                        pt = tp2ps.tile([P, P], f16, tag="tp2_ps",
                                        name="tp2_ps")
                        nc.tensor.transpose(pt[:], x2t[:, ds(P * j, P)],
                                            ident_h[:])
                        nc.vector.tensor_copy(x2T[j][:, ds(P * i, P)], pt[:])
            s_aoT.close()
            s_woh.close()
            s_kv.close()

            # ===== phase E: MLP per token group =====
            hm_p = s_hm.enter_context(
                tc.tile_pool(name="hm", bufs=1, side="right"))
            hm = [hm_p.tile([P, GS], f16, tag=f"hm{m}", name=f"hm{m}")
                  for m in range(MC)]
            w2sb_p = s_w2.enter_context(tc.tile_pool(name="w2sb", bufs=1))
            w2sb = []
            for m in range(MC):
                wf = w2sb_p.tile([P, D], f32, tag="w2_f", name="w2_f", bufs=3)
                nc.scalar.dma_start(wf[:], ap["m2w"][ds(P * m, P), :])
                t = w2sb_p.tile([P, D], f8, tag=f"w2_{m}", name=f"w2_{m}")
                nc.gpsimd.tensor_scalar(t[:], wf[:], 16.0, None, OP.mult)
                w2sb.append(t)
            with (
                tc.tile_pool(name="e_sb", bufs=2) as esb,
                tc.tile_pool(name="m1_ps", bufs=2, space="PSUM") as m1ps,
                tc.tile_pool(name="m2_ps", bufs=1, space="PSUM") as m2ps,
            ):
                for g in range(NG):
                    for m in range(MC):
                        pt = m1ps.tile([P, GS], f32, tag="m1p", name="m1p")
                        for j in range(DC):
                            nc.tensor.matmul(pt[:], w1sb[j][:, ds(P * m, P)],
                                             x2T[j][:, ds(GS * g, GS)],
                                             start=(j == 0), stop=(j == DC - 1))
                        nc.scalar.activation(hm[m][:], pt[:], AF.Gelu,
                                             bias=m1b_c[:, m:m + 1],
                                             scale=0.0625)
                    for half in range(2):
                        m2p = [m2ps.tile([P, GS], f32, tag=f"m2p{ss}",
                                         name=f"m2p{ss}") for ss in range(4)]
                        for m in range(MC):
                            for ss in range(4):
                                nc.tensor.matmul(
                                    m2p[ss][:], hm[m][:, ds(P * ss, P)],
                                    w2sb[m][:, ds(GS * half, GS)],
                                    start=(m == 0), stop=(m == MC - 1))
                        for ss in range(4):
                            i = 4 * g + ss
                            rt = esb.tile([P, GS], f32, tag="res_a",
                                          name="res_a")
                            nc.sync.dma_start(
                                rt[:], attn_sc[i][:, ds(GS * half, GS)])
                            tm = esb.tile([P, GS], f32, tag="e_tmp",
                                          name="e_tmp")
                            nc.vector.tensor_scalar(tm[:], m2p[ss][:], 0.0625,
                                                    None, OP.mult)
                            tm2 = esb.tile([P, GS], f32, tag="e_tmp2",
                                           name="e_tmp2")
                            nc.vector.tensor_add(tm2[:], tm[:],
                                                 m2b_b[:, ds(GS * half, GS)])
                            ot = esb.tile([P, GS], f32, tag="e_out",
                                          name="e_out")
                            nc.vector.tensor_add(ot[:], tm2[:], rt[:])
                            nc.sync.dma_start(
                                out[ds(P * i, P), ds(GS * half, GS)], ot[:])
        finally:
            for s in (s_w2, s_w1, s_x2T, s_qT, s_x1n, s_woh, s_hm, s_aoT,
                      s_pef, s_kv):
                s.close()


def kernel(**inputs):
    nc = build()
    x = np.ascontiguousarray(inputs["x"], dtype=np.float32)
    z = np.ascontiguousarray(inputs["z"], dtype=np.float32)
    base = {}
    for nm, _, _ in W2D + W2DR:
        base[nm] = np.ascontiguousarray(inputs[nm], dtype=np.float32)
    for nm, _ in W1D:
        base[nm] = np.ascontiguousarray(inputs[nm], dtype=np.float32)
    in_maps = []
    for c in range(B):
        m = dict(base)
        m["x"] = x[c]
        m["z"] = z[c:c + 1]
        in_maps.append(m)
    res = run_bass_kernel_spmd(nc, in_maps, list(range(B)))
    _cache["last"] = res
    return np.stack([res.results[c]["out"] for c in range(B)], axis=0)


# revision 14
# speedup vs baseline: 1.5541x; 1.5541x over previous
"""DiT block (Linformer attention + adaLN + MLP) on 8 TRN2 NeuronCores.

Sharding: data-parallel over batch (B=8 -> one batch element per core).

v2 layout (per core, S=2048 tokens, D=1024 features):
 - Phase A conditioning matvecs run in f32r (single-pass fp32 at full PE
   rate for moving dims >= 256); scale/offset rows are broadcast to 128
   partitions with gpsimd.partition_broadcast instead of PE ones-matmuls.
 - All other weights are converted once to f16 via DRAM->DRAM casting DMAs
   on the gpsimd (SWDGE) queue, then streamed as plain f16 HWDGE loads:
   no per-use casts, and the MLP weights are no longer re-streamed per
   token group (m1w resident in SBUF, m2w streamed f16 once per group).
 - Attention: per half-group (8 heads) all scores matmuls are issued as one
   64-row-tiled block, exps on scalar, then all attn@V matmuls; softmax
   denominators come from a fused ones-column appended to v_proj, and the
   1/denom normalization uses gpsimd partition_broadcast + vector
   reciprocal + psum multiply (no PE broadcast matmuls).
 - wo+residual is fused with adaLN2 + transposes per 128-token chunk, so
   attn_out is written to DRAM once and only re-read for the final
   residual add.

DMA queues: sync carries x/attn_sc/out streams and the f16 attention
weights; scalar carries phase-A f32r weights, EF/w1/w2 f16 streams and
bias rows; gpsimd (SWDGE) carries the 8 one-time casts plus all
latency-chained tiny transfers (h->cols, partition broadcasts) so they
never block a bulk queue at its head.
"""
import contextlib

import numpy as np

import concourse.bass as bass
import concourse.mybir as mybir
import concourse.tile as tile
from concourse import bacc
from concourse.bass import ds, ts
from concourse.bass_utils import run_bass_kernel_spmd
from concourse.masks import make_identity

f32 = mybir.dt.float32
f32r = mybir.dt.float32r
f16 = mybir.dt.float16
f8 = mybir.dt.float8e4
AF = mybir.ActivationFunctionType
OP = mybir.AluOpType

B, S, D, H, K, MLP, ZD = 8, 2048, 1024, 16, 256, 4096, 1024
DH = D // H      # 64
P = 128
SC = S // P      # 16 token chunks of 128
DC = D // P      # 8 feature chunks of 128
NG = 4           # token groups of 512
GS = 512
MC = MLP // P    # 32
KC = K // P      # 2
EPS = 1e-6

W2D = [("wq", D, D), ("wk", D, D), ("wv", D, D), ("wo", D, D),
       ("Ew", S, K), ("Fw", S, K),
       ("m1w", D, MLP), ("m2w", MLP, D)]
W2DR = [("h1w", ZD, D), ("g1w", D, D), ("be1w", D, D),
        ("h2w", ZD, D), ("g2w", D, D), ("be2w", D, D)]
W1D = [("bq", D), ("bk", D), ("bv", D), ("bo", D), ("Eb", K), ("Fb", K),
       ("h1b", D), ("g1b", D), ("be1b", D), ("h2b", D), ("g2b", D), ("be2b", D),
       ("m1b", MLP), ("m2b", D)]

_cache = {}


def build():
    if "nc" in _cache:
        return _cache["nc"]
    nc = bacc.Bacc("TRN2", target_bir_lowering=False, debug=False, num_devices=8)
    ap = {}
    ap["x"] = nc.dram_tensor("x", [S, D], f32, kind="ExternalInput").ap()
    ap["z"] = nc.dram_tensor("z", [1, ZD], f32r, kind="ExternalInput").ap()
    for nm, a, b in W2D:
        ap[nm] = nc.dram_tensor(nm, [a, b], f32, kind="ExternalInput").ap()
    for nm, a, b in W2DR:
        ap[nm] = nc.dram_tensor(nm, [a, b], f32r, kind="ExternalInput").ap()
    for nm, a in W1D:
        ap[nm] = nc.dram_tensor(nm, [a], f32, kind="ExternalInput").ap()
    out = nc.dram_tensor("out", [S, D], f32, kind="ExternalOutput").ap()
    with tile.TileContext(nc, trace_sim=False) as tc:
        _emit(nc, tc, ap, out)
    nc.compile()
    _cache["nc"] = nc
    return nc


def _emit(nc, tc, ap, out):
    ctx = contextlib.ExitStack()
    with ctx:
        # ---------- whole-kernel pools ----------
        const = ctx.enter_context(tc.tile_pool(name="const", bufs=1))
        cols = ctx.enter_context(tc.tile_pool(name="cols", bufs=1))
        bc = ctx.enter_context(tc.tile_pool(name="bc", bufs=1))
        dram = ctx.enter_context(tc.tile_pool(name="dram", bufs=1, space="DRAM"))

        attn_sc = [dram.tile([P, D], f32, tag=f"attn_sc{i}", name=f"attn_sc{i}")
                   for i in range(SC)]

        ident_f = const.tile([P, P], f32, tag="ident_f", name="ident_f")
        make_identity(nc, ident_f)
        ident_h = const.tile([P, P], f16, tag="ident_h", name="ident_h")
        nc.vector.tensor_copy(ident_h[:], ident_f[:])
        eps_t = const.tile([P, 1], f32, tag="eps", name="eps")
        nc.vector.memset(eps_t[:], EPS)
        ones_f = const.tile([P, 1], f32, tag="ones_f", name="ones_f")
        nc.vector.memset(ones_f[:], 1.0)
        onescol_h = const.tile([P, 1], f16, tag="onescol_h", name="onescol_h")
        nc.vector.tensor_copy(onescol_h[:], ones_f[:])

        def col_load(name, n):
            """1-D DRAM vector [n*128] -> sbuf [128, n] (partition-major)."""
            t = cols.tile([P, n], f32, tag=f"cols_{name}", name=f"cols_{name}")
            for j in range(n):
                nc.scalar.dma_start(t[:, j:j + 1], ap[name][ds(P * j, P)])
            return t

        # broadcast result tiles (f16, whole-kernel)
        scale1_b = bc.tile([P, D], f16, tag="scale1_b", name="scale1_b")
        offset1_b = bc.tile([P, D], f16, tag="offset1_b", name="offset1_b")
        scale2_b = bc.tile([P, D], f16, tag="scale2_b", name="scale2_b")
        offset2_b = bc.tile([P, D], f16, tag="offset2_b", name="offset2_b")
        bo_b = bc.tile([P, D], f16, tag="bo_b", name="bo_b")
        m2b_b = bc.tile([P, D], f16, tag="m2b_b", name="m2b_b")

        zc_f = cols.tile([P, DC], f32r, tag="zc_f", name="zc_f")
        for j in range(DC):
            nc.scalar.dma_start(zc_f[:, j:j + 1], ap["z"][0:1, ds(P * j, P)])

        def vec_layer(vsb, vps, rowp, wname, lhs_cols, bias_row, act, out_row):
            """out_row[1, D] = act(lhs^T @ w + bias) with f32r weights."""
            pts = [vps.tile([1, GS], f32, tag=f"vps{h}", name=f"vps{h}")
                   for h in range(2)]
            for j in range(DC):
                wt = vsb.tile([P, D], f32r, tag=f"vw_{wname}", name=f"vw_{wname}",
                              bufs=4)
                nc.scalar.dma_start(wt[:], ap[wname][ds(P * j, P), :])
                for h in range(2):
                    nc.tensor.matmul(pts[h][:], lhs_cols[:, j:j + 1],
                                     wt[:, ds(GS * h, GS)],
                                     start=(j == 0), stop=(j == DC - 1))
            for h in range(2):
                pre = rowp.tile([1, GS], f32, tag=f"vpre{h}", name=f"vpre{h}",
                                bufs=2)
                nc.vector.tensor_add(pre[:], pts[h][:],
                                     bias_row[0:1, ds(GS * h, GS)])
                if act is None:
                    nc.vector.tensor_copy(out_row[0:1, ds(GS * h, GS)], pre[:])
                else:
                    nc.scalar.activation(out_row[0:1, ds(GS * h, GS)],
                                         pre[:], act)

        def a_chain(vsb, vps, rowp, hw, hb, gw, gb, bw, bb, tagn, sc_b, of_b):
            def row_load(name):
                t = rowp.tile([1, D], f32, tag="arow_b", name=f"row_{name}",
                              bufs=2)
                nc.scalar.dma_start(t[:], ap[name][0:D])
                return t

            hb_row = row_load(hb)
            h_row = rowp.tile([1, D], f32, tag="h_row", name=f"h_{tagn}")
            vec_layer(vsb, vps, rowp, hw, zc_f, hb_row, AF.Silu, h_row)
            h_row_r = rowp.tile([1, D], f32r, tag="h_row_r", name=f"hr_{tagn}")
            nc.vector.tensor_copy(h_row_r[:], h_row[:])
            h_c = cols.tile([P, DC], f32r, tag=f"c_{tagn}", name=f"c_{tagn}")
            for j in range(DC):
                nc.gpsimd.dma_start(h_c[:, j:j + 1], h_row_r[0:1, ds(P * j, P)])
            gb_row = row_load(gb)
            sc_row = rowp.tile([1, D], f32, tag="sc_row", name=f"sc_{tagn}")
            vec_layer(vsb, vps, rowp, gw, h_c, gb_row, None, sc_row)
            sc_row_h = rowp.tile([1, D], f16, tag="sc_row_h", name=f"sch_{tagn}")
            nc.vector.tensor_copy(sc_row_h[:], sc_row[:])
            nc.gpsimd.partition_broadcast(sc_b[:], sc_row_h[0:1, :])
            bb_row = row_load(bb)
            of_row = rowp.tile([1, D], f32, tag="of_row", name=f"of_{tagn}")
            vec_layer(vsb, vps, rowp, bw, h_c, bb_row, None, of_row)
            of_row_h = rowp.tile([1, D], f16, tag="of_row_h", name=f"ofh_{tagn}")
            nc.vector.tensor_copy(of_row_h[:], of_row[:])
            nc.gpsimd.partition_broadcast(of_b[:], of_row_h[0:1, :])

        # =========== phase A1: adaLN1 conditioning vectors (f32r) ===========
        with (
            tc.tile_pool(name="vec1_sb", bufs=1) as vsb1,
            tc.tile_pool(name="row1_sb", bufs=1) as rowp1,
            tc.tile_pool(name="vec1_ps", bufs=2, space="PSUM") as vps1,
        ):
            a_chain(vsb1, vps1, rowp1, "h1w", "h1b", "g1w", "g1b",
                    "be1w", "be1b", "h1", scale1_b, offset1_b)
        # manual pool stacks (LIFO per SBUF side)
        s_woh = contextlib.ExitStack()   # left (created first: popped last)
        s_qT = contextlib.ExitStack()    # left
        s_x1n = contextlib.ExitStack()   # left
        s_x2T = contextlib.ExitStack()   # left
        s_w1 = contextlib.ExitStack()    # left
        s_w2 = contextlib.ExitStack()    # left (E only)
        s_kv = contextlib.ExitStack()    # right (kpT/vpe, lingers under aoT)
        s_pef = contextlib.ExitStack()   # right (pefE/F + colsums, popped at KV end)
        s_aoT = contextlib.ExitStack()   # right
        s_hm = contextlib.ExitStack()    # right
        try:
            # ===== phase B: adaLN1 + transposes + qT =====
            qT_p = s_qT.enter_context(tc.tile_pool(name="qT", bufs=1))
            qT = [[qT_p.tile([P, GS], f16, tag=f"qT_{j}_{g}", name=f"qT_{j}_{g}")
                   for g in range(NG)] for j in range(DC)]
            x1n_p = s_x1n.enter_context(tc.tile_pool(name="x1nat", bufs=1))
            x1n = []
            with (
                tc.tile_pool(name="wq_sb", bufs=1) as wqsb,
                tc.tile_pool(name="ln1_sb", bufs=2) as ln_sb,
                tc.tile_pool(name="x1Trot", bufs=1) as x1t_p,
                tc.tile_pool(name="tp1_ps", bufs=2, space="PSUM") as ln_ps,
                tc.tile_pool(name="q_ps", bufs=3, space="PSUM") as qps,
            ):
                wq_r = []
                for j in range(DC):
                    wf = wqsb.tile([P, D], f32, tag="wq_f", name="wq_f",
                                   bufs=2)
                    nc.scalar.dma_start(wf[:], ap["wq"][ds(P * j, P), :])
                    wr = wqsb.tile([P, D], f16, tag=f"wq_r{j}", name=f"wq_r{j}")
                    nc.scalar.copy(wr[:], wf[:])
                    wq_r.append(wr)
                bq_c = col_load("bq", DC)
                bk_c = col_load("bk", DC)
                Fb_c = col_load("Fb", KC)
                for g in range(NG):
                    x1T_g = [x1t_p.tile([P, GS], f16, tag=f"x1T_{j}",
                                        name=f"x1T_{j}") for j in range(DC)]
                    for ii in range(4):
                        i = 4 * g + ii
                        xt = ln_sb.tile([P, D], f32, tag="ln_in",
                                        name=f"ln_in{i}", bufs=4)
                        nc.sync.dma_start(xt[:], ap["x"][ds(P * i, P), :])
                        st = ln_sb.tile([P, 2, 6], f32, tag="ln_st",
                                        name="ln_st")
                        nc.vector.bn_stats(st[:, 0, :], xt[:, 0:GS])
                        nc.vector.bn_stats(st[:, 1, :], xt[:, GS:D])
                        mv = ln_sb.tile([P, 2], f32, tag="ln_mv", name="ln_mv")
                        nc.vector.bn_aggr(mv[:], st[:])
                        sd = ln_sb.tile([P, 1], f32, tag="ln_sd",
                                        name="ln_sd")
                        nc.scalar.activation(sd[:], mv[:, 1:2], AF.Sqrt,
                                             bias=eps_t[:])
                        rstd = ln_sb.tile([P, 1], f32, tag="ln_rstd",
                                          name="ln_rstd")
                        nc.vector.reciprocal_approx_fast(rstd[:], sd[:])
                        nmr = ln_sb.tile([P, 1], f32, tag="ln_nmr",
                                         name="ln_nmr")
                        nc.vector.tensor_scalar(nmr[:], mv[:, 0:1], rstd[:],
                                                -1.0, OP.mult, OP.mult)
                        xn = ln_sb.tile([P, D], f32, tag="ln_xn", name="ln_xn")
                        nc.scalar.activation(xn[:], xt[:], AF.Identity,
                                             bias=nmr[:], scale=rstd[:])
                        nc.gpsimd.tensor_mul(xn[:], xn[:], scale1_b[:])
                        x1t = x1n_p.tile([P, D], f16, tag=f"nat{i}",
                                         name=f"nat{i}")
                        nc.vector.tensor_add(x1t[:], xn[:], offset1_b[:])
                        x1n.append(x1t)
                        for j in range(DC):
                            pt = ln_ps.tile([P, P], f16, tag="tp_ps",
                                            name="tp_ps")
                            nc.tensor.transpose(pt[:], x1t[:, ds(P * j, P)],
                                                ident_h[:])
                            nc.vector.tensor_copy(
                                x1T_g[j][:, ds(P * ii, P)], pt[:])
                    for jo in range(DC):
                        pt = qps.tile([P, GS], f32, tag="q_ps", name="q_ps")
                        for j in range(DC):
                            nc.tensor.matmul(pt[:],
                                             wq_r[j][:, ds(P * jo, P)],
                                             x1T_g[j][:],
                                             start=(j == 0),
                                             stop=(j == DC - 1))
                        nc.scalar.activation(qT[jo][g][:], pt[:], AF.Identity,
                                             bias=bq_c[:, jo:jo + 1])

            # ===== phase B2: P_EF = x1^T @ [Ew|Fw], colsums fused after =====
            kv_sb = s_kv.enter_context(
                tc.tile_pool(name="kv_sb", bufs=1, side="right"))
            pef_sb = s_pef.enter_context(
                tc.tile_pool(name="pef_sb", bufs=1, side="right"))
            pefE = [pef_sb.tile([P, K], f16, tag=f"pefE{j}", name=f"pefE{j}")
                    for j in range(DC)]
            pefF = [pef_sb.tile([P, K], f16, tag=f"pefF{j}", name=f"pefF{j}")
                    for j in range(DC)]
            cs_row = pef_sb.tile([1, 2 * K], f32, tag="cs", name="cs")
            with tc.tile_pool(name="ef_sb", bufs=1) as efsb:
                ef_h = []
                for i in range(SC):
                    ff = efsb.tile([P, 2 * K], f32, tag="ef_f", name="ef_f",
                                   bufs=3)
                    nc.scalar.dma_start(ff[:, 0:K], ap["Ew"][ds(P * i, P), :])
                    nc.scalar.dma_start(ff[:, K:2 * K],
                                        ap["Fw"][ds(P * i, P), :])
                    t = efsb.tile([P, 2 * K], f16, tag=f"ef{i}", name=f"ef{i}")
                    nc.scalar.copy(t[:], ff[:])
                    ef_h.append(t)
                m1b_c = col_load("m1b", MC)
                with tc.tile_pool(name="pef_ps", bufs=1, space="PSUM") as pfps:
                    pef_ps = [pfps.tile([P, 2 * K], f32, tag=f"pefp{j}",
                                        name=f"pefp{j}") for j in range(DC)]
                    for i in range(SC):
                        for j in range(DC):
                            nc.tensor.matmul(pef_ps[j][:],
                                             x1n[i][:, ds(P * j, P)],
                                             ef_h[i][:],
                                             start=(i == 0), stop=(i == SC - 1))
                    for j in range(DC):
                        nc.scalar.copy(pefE[j][:], pef_ps[j][:, 0:K])
                        nc.scalar.copy(pefF[j][:], pef_ps[j][:, K:2 * K])
                with tc.tile_pool(name="cs_ps", bufs=1, space="PSUM") as csps:
                    cs_ps = csps.tile([1, 2 * K], f32, tag="cs_ps",
                                      name="cs_ps")
                    for i in range(SC):
                        nc.tensor.matmul(cs_ps[:], onescol_h[:], ef_h[i][:],
                                         start=(i == 0), stop=(i == SC - 1))
                    nc.vector.tensor_copy(cs_row[:], cs_ps[:])
            s_x1n.close()

            # ===== phase KV: k_projT, v_proj_ext =====
            kpT = [kv_sb.tile([P, K], f16, tag=f"kpT{j}", name=f"kpT{j}")
                   for j in range(DC)]
            vpe = [kv_sb.tile([P, 65 * H], f16, tag=f"vpe{c}", name=f"vpe{c}")
                   for c in range(KC)]
            with (
                tc.tile_pool(name="kv_w", bufs=1) as kvw,
                tc.tile_pool(name="kv_bias", bufs=1) as kvb,
                tc.tile_pool(name="kv_tmp", bufs=2) as kvt,
                tc.tile_pool(name="kv_ps", bufs=2, space="PSUM") as kvps,
            ):
                Eb_row = kvb.tile([1, K], f32, tag="Eb_row", name="Eb_row")
                nc.scalar.dma_start(Eb_row[:], ap["Eb"][0:K])
                Eb_b = kvb.tile([P, K], f32, tag="Eb_b", name="Eb_b")
                nc.gpsimd.partition_broadcast(Eb_b[:], Eb_row[0:1, :])
                csE_b = kvb.tile([P, K], f32, tag="csE_b", name="csE_b")
                nc.gpsimd.partition_broadcast(csE_b[:], cs_row[0:1, 0:K])
                bv_row = kvb.tile([1, D], f32, tag="bv_row", name="bv_row")
                nc.scalar.dma_start(bv_row[:], ap["bv"][0:D])
                bv_b = kvb.tile([P, D], f32, tag="bv_b", name="bv_b")
                nc.gpsimd.partition_broadcast(bv_b[:], bv_row[0:1, :])
                csF_c = kvb.tile([P, KC], f32, tag="csF_c", name="csF_c")
                for c in range(KC):
                    nc.gpsimd.dma_start(csF_c[:, c:c + 1],
                                        cs_row[0:1, ds(K + P * c, P)])
                kp_bias = []
                for j in range(DC):
                    bt = kvb.tile([P, K], f32, tag=f"kpb{j}", name=f"kpb{j}")
                    nc.vector.tensor_scalar(bt[:], csE_b[:], bk_c[:, j:j + 1],
                                            None, OP.mult)
                    nc.vector.tensor_add(bt[:], bt[:], Eb_b[:])
                    kp_bias.append(bt)
                vp_bias = []
                for c in range(KC):
                    bt = kvb.tile([P, D], f32, tag=f"vpb{c}", name=f"vpb{c}")
                    nc.vector.tensor_scalar(bt[:], bv_b[:], csF_c[:, c:c + 1],
                                            Fb_c[:, c:c + 1], OP.mult, OP.add)
                    vp_bias.append(bt)

                wk_r, wv_r = [], []
                for j in range(DC):
                    for nm, lst, tg in (("wk", wk_r, "wk"), ("wv", wv_r, "wv")):
                        wf = kvw.tile([P, D], f32, tag=f"{tg}_f",
                                      name=f"{tg}_f", bufs=2)
                        nc.sync.dma_start(wf[:], ap[nm][ds(P * j, P), :])
                        wr = kvw.tile([P, D], f16, tag=f"{tg}_r{j}",
                                      name=f"{tg}_r{j}")
                        nc.scalar.copy(wr[:], wf[:])
                        lst.append(wr)
                for jo in range(DC):
                    pt = kvps.tile([P, K], f32, tag="kp_ps", name="kp_ps")
                    for j in range(DC):
                        nc.tensor.matmul(pt[:], wk_r[j][:, ds(P * jo, P)],
                                         pefE[j][:],
                                         start=(j == 0), stop=(j == DC - 1))
                    nc.vector.tensor_add(kpT[jo][:], pt[:], kp_bias[jo][:])
                for hf in range(2):
                    for c in range(KC):
                        pt = kvps.tile([P, GS], f32, tag="vp_ps", name="vp_ps")
                        for j in range(DC):
                            nc.tensor.matmul(pt[:], pefF[j][:, ds(P * c, P)],
                                             wv_r[j][:, ds(GS * hf, GS)],
                                             start=(j == 0), stop=(j == DC - 1))
                        tmp = kvt.tile([P, GS], f32, tag="vp_tmp",
                                       name="vp_tmp")
                        nc.vector.tensor_add(tmp[:], pt[:],
                                             vp_bias[c][:, ds(GS * hf, GS)])
                        for hh in range(8):
                            h = 8 * hf + hh
                            nc.vector.tensor_copy(vpe[c][:, ds(65 * h, 64)],
                                                  tmp[:, ds(64 * hh, 64)])
                for c in range(KC):
                    for h in range(H):
                        nc.vector.tensor_copy(vpe[c][:, ds(65 * h + 64, 1)],
                                              ones_f[:, 0:1])

            s_pef.close()

            # ===== phase A2: adaLN2 conditioning vectors =====
            with (
                tc.tile_pool(name="vec2_sb", bufs=1) as vsb2,
                tc.tile_pool(name="row2_sb", bufs=1) as rowp2,
                tc.tile_pool(name="vec2_ps", bufs=2, space="PSUM") as vps2,
            ):
                a_chain(vsb2, vps2, rowp2, "h2w", "h2b", "g2w", "g2b",
                        "be2w", "be2b", "h2", scale2_b, offset2_b)
                brow = rowp2.tile([1, D], f32, tag="brow", name="bo_row",
                                  bufs=2)
                nc.scalar.dma_start(brow[:], ap["bo"][0:D])
                browh = rowp2.tile([1, D], f16, tag="browh", name="bo_row_h",
                                   bufs=2)
                nc.vector.tensor_copy(browh[:], brow[:])
                nc.gpsimd.partition_broadcast(bo_b[:], browh[0:1, :])
                brow2 = rowp2.tile([1, D], f32, tag="brow", name="m2b_row",
                                   bufs=2)
                nc.scalar.dma_start(brow2[:], ap["m2b"][0:D])
                brow2h = rowp2.tile([1, D], f16, tag="browh", name="m2b_row_h",
                                    bufs=2)
                nc.vector.tensor_copy(brow2h[:], brow2[:])
                nc.gpsimd.partition_broadcast(m2b_b[:], brow2h[0:1, :])

            # ===== load wo (f16) for C3 =====
            woh_sb = s_woh.enter_context(
                tc.tile_pool(name="woh_sb", bufs=1, side="right"))
            wo_r = []
            for j in range(DC):
                wf = woh_sb.tile([P, D], f32, tag="wo_f", name="wo_f", bufs=2)
                nc.sync.dma_start(wf[:], ap["wo"][ds(P * j, P), :])
                wr = woh_sb.tile([P, D], f16, tag=f"wo_r{j}", name=f"wo_r{j}")
                nc.vector.tensor_copy(wr[:], wf[:])
                wo_r.append(wr)

            # ===== phase C2: attention =====
            aoT_p = s_aoT.enter_context(
                tc.tile_pool(name="aoT", bufs=1, side="right"))
            aoT = [[aoT_p.tile([P, GS], f16, tag=f"aoT_{j}_{g}",
                               name=f"aoT_{j}_{g}")
                    for g in range(NG)] for j in range(DC)]
            with (
                tc.tile_pool(name="exp_sb", bufs=1) as expsb,
                tc.tile_pool(name="nrm_sb", bufs=1) as nrmsb,
                tc.tile_pool(name="sc_ps", bufs=4, space="PSUM") as scps,
                tc.tile_pool(name="av_ps", bufs=3, space="PSUM") as avps,
            ):
                def emit_scores_half(hg):
                    """scores+exp for the 8 heads of half-group hg (0..7)."""
                    g, hb = hg // 2, (hg % 2) * 8
                    exps = []
                    for ph in range(4):
                        for e in range(2):
                            h = hb + 2 * ph + e
                            j, r0 = h // 2, 64 * (h % 2)
                            ets = []
                            for c in range(KC):
                                spt = scps.tile([P, GS], f32, tag="sc",
                                                name="sc")
                                nc.tensor.matmul(
                                    spt[:],
                                    kpT[j][r0:r0 + 64, ds(P * c, P)],
                                    qT[j][g][r0:r0 + 64, :],
                                    start=True, stop=True)
                                et = expsb.tile([P, GS], f16, tag="exp",
                                                name="exp", bufs=34)
                                nc.scalar.activation(et[:], spt[:], AF.Exp,
                                                     scale=0.125)
                                ets.append(et)
                            exps.append((h, ets))
                    return exps

                def emit_av_half(exps, g):
                    for h, ets in exps:
                        j, r0 = h // 2, 64 * (h % 2)
                        apt = avps.tile([65, GS], f32, tag="av", name="av")
                        for c in range(KC):
                            nc.tensor.matmul(apt[:], vpe[c][:, ds(65 * h, 65)],
                                             ets[c][:],
                                             start=(c == 0), stop=(c == KC - 1))
                        den = nrmsb.tile([1, GS], f32, tag="den", name="den",
                                         bufs=4)
                        nc.vector.tensor_copy(den[:], apt[64:65, :])
                        den_b = nrmsb.tile([64, GS], f32, tag="den_b",
                                           name="den_b", bufs=3)
                        nc.gpsimd.partition_broadcast(den_b[:], den[0:1, :])
                        rec = nrmsb.tile([64, GS], f32, tag="rec", name="rec",
                                         bufs=3)
                        nc.vector.reciprocal_approx_fast(rec[:], den_b[:])
                        nc.vector.tensor_mul(aoT[j][g][r0:r0 + 64, :],
                                             apt[0:64, :], rec[:])

                prev = None
                for hg in range(2 * NG):
                    cur = emit_scores_half(hg)
                    if prev is not None:
                        emit_av_half(*prev)
                    prev = (cur, hg // 2)
                emit_av_half(*prev)
            s_qT.close()

            # ===== phase C3: wo + residual + adaLN2 + transposes, fused =====
            x2T_p = s_x2T.enter_context(tc.tile_pool(name="x2T", bufs=1))
            x2T = [x2T_p.tile([P, S], f16, tag=f"x2T_{j}", name=f"x2T_{j}")
                   for j in range(DC)]
            with (
                tc.tile_pool(name="c3_sb", bufs=1) as c3sb,
                tc.tile_pool(name="wo_ps", bufs=3, space="PSUM") as wops,
                tc.tile_pool(name="tp2_ps", bufs=2, space="PSUM") as tp2ps,
            ):
                for i in range(SC):
                    g, c = i // 4, (i % 4) * P
                    xt = c3sb.tile([P, D], f32, tag="res_x", name="res_x",
                                   bufs=2)
                    nc.scalar.dma_start(xt[:], ap["x"][ds(P * i, P), :])
                    at = c3sb.tile([P, D], f32, tag="attn_nat",
                                   name="attn_nat", bufs=2)
                    for hf in range(2):
                        pt = wops.tile([P, GS], f32, tag="wo_ps", name="wo_ps")
                        for j in range(DC):
                            nc.tensor.matmul(pt[:], aoT[j][g][:, ds(c, P)],
                                             wo_r[j][:, ds(GS * hf, GS)],
                                             start=(j == 0), stop=(j == DC - 1))
                        tm = c3sb.tile([P, GS], f32, tag="wo_tmp",
                                       name="wo_tmp", bufs=2)
                        nc.vector.tensor_add(tm[:], pt[:],
                                             bo_b[:, ds(GS * hf, GS)])
                        nc.vector.tensor_add(at[:, ds(GS * hf, GS)], tm[:],
                                             xt[:, ds(GS * hf, GS)])
                    nc.sync.dma_start(attn_sc[i][:], at[:])
                    st = c3sb.tile([P, 2, 6], f32, tag="ln2_st",
                                   name="ln2_st", bufs=2)
                    nc.vector.bn_stats(st[:, 0, :], at[:, 0:GS])
                    nc.vector.bn_stats(st[:, 1, :], at[:, GS:D])
                    mv = c3sb.tile([P, 2], f32, tag="ln2_mv", name="ln2_mv")
                    nc.vector.bn_aggr(mv[:], st[:])
                    sd = c3sb.tile([P, 1], f32, tag="ln2_sd",
                                   name="ln2_sd")
                    nc.scalar.activation(sd[:], mv[:, 1:2], AF.Sqrt,
                                         bias=eps_t[:])
                    rstd = c3sb.tile([P, 1], f32, tag="ln2_rstd",
                                     name="ln2_rstd")
                    nc.vector.reciprocal_approx_fast(rstd[:], sd[:])
                    nmr = c3sb.tile([P, 1], f32, tag="ln2_nmr", name="ln2_nmr")
                    nc.vector.tensor_scalar(nmr[:], mv[:, 0:1], rstd[:],
                                            -1.0, OP.mult, OP.mult)
                    xn = c3sb.tile([P, D], f32, tag="ln2_xn", name="ln2_xn")
                    nc.scalar.activation(xn[:], at[:], AF.Identity,
                                         bias=nmr[:], scale=rstd[:])
                    nc.gpsimd.tensor_mul(xn[:], xn[:], scale2_b[:])
                    x2t = c3sb.tile([P, D], f16, tag="x2nat", name="x2nat",
                                    bufs=2)
                    nc.vector.tensor_add(x2t[:], xn[:], offset2_b[:])
                    for j in range(DC):
                        pt = tp2ps.tile([P, P], f16, tag="tp2_ps",
                                        name="tp2_ps")
                        nc.tensor.transpose(pt[:], x2t[:, ds(P * j, P)],
                                            ident_h[:])
                        nc.vector.tensor_copy(x2T[j][:, ds(P * i, P)], pt[:])
            s_aoT.close()
            s_woh.close()
            s_kv.close()

            # ===== phase E: MLP per token group =====
            hm_p = s_hm.enter_context(
                tc.tile_pool(name="hm", bufs=1, side="right"))
            hm = [hm_p.tile([P, GS], f16, tag=f"hm{m}", name=f"hm{m}")
                  for m in range(MC)]
            # w2 resident f16, filled at E start (f32 stream + gpsimd cast)
            w2sb_p = s_w2.enter_context(tc.tile_pool(name="w2sb", bufs=1))
            w2sb = []
            for m in range(MC):
                wf = w2sb_p.tile([P, D], f32, tag="w2_f", name="w2_f", bufs=3)
                nc.scalar.dma_start(wf[:], ap["m2w"][ds(P * m, P), :])
                t = w2sb_p.tile([P, D], f16, tag=f"w2_{m}", name=f"w2_{m}")
                nc.gpsimd.tensor_copy(t[:], wf[:])
                w2sb.append(t)
            # w1 streamed f32 per (group, m-quarter, j) + inline gpsimd cast;
            # m1 runs in m-quarter passes so psum for m1 and m2 share 8 banks
            QW = MLP // 4  # 1024 m-columns per quarter
            with (
                tc.tile_pool(name="w1st", bufs=1) as w1st,
                tc.tile_pool(name="e_sb", bufs=2) as esb,
                tc.tile_pool(name="mlp_ps", bufs=1, space="PSUM") as mlpps,
            ):
                for g in range(NG):
                    for q in range(4):
                        pts = [mlpps.tile([P, GS], f32, tag=f"p{mm}",
                                          name=f"p{mm}") for mm in range(8)]
                        for j in range(DC):
                            wf = w1st.tile([P, QW], f32, tag="w1_f",
                                           name="w1_f", bufs=2)
                            nc.sync.dma_start(
                                wf[:], ap["m1w"][ds(P * j, P), ds(QW * q, QW)])
                            wt = w1st.tile([P, QW], f16, tag="w1_h",
                                           name="w1_h", bufs=2)
                            nc.gpsimd.tensor_copy(wt[:], wf[:])
                            for mm in range(8):
                                nc.tensor.matmul(
                                    pts[mm][:], wt[:, ds(P * mm, P)],
                                    x2T[j][:, ds(GS * g, GS)],
                                    start=(j == 0), stop=(j == DC - 1))
                        for mm in range(8):
                            m = 8 * q + mm
                            nc.scalar.activation(hm[m][:], pts[mm][:], AF.Gelu,
                                                 bias=m1b_c[:, m:m + 1])
                    for half in range(2):
                        m2p = [mlpps.tile([P, GS], f32, tag=f"p{ss}",
                                          name=f"m2p{ss}") for ss in range(4)]
                        for m in range(MC):
                            for ss in range(4):
                                nc.tensor.matmul(
                                    m2p[ss][:], hm[m][:, ds(P * ss, P)],
                                    w2sb[m][:, ds(GS * half, GS)],
                                    start=(m == 0), stop=(m == MC - 1))
                        for ss in range(4):
                            i = 4 * g + ss
                            rt = esb.tile([P, GS], f32, tag="res_a",
                                          name="res_a")
                            nc.sync.dma_start(
                                rt[:], attn_sc[i][:, ds(GS * half, GS)])
                            tm = esb.tile([P, GS], f32, tag="e_tmp",
                                          name="e_tmp")
                            nc.vector.tensor_add(tm[:], m2p[ss][:],
                                                 m2b_b[:, ds(GS * half, GS)])
                            ot = esb.tile([P, GS], f32, tag="e_out",
                                          name="e_out")
                            nc.vector.tensor_add(ot[:], tm[:], rt[:])
                            nc.sync.dma_start(
                                out[ds(P * i, P), ds(GS * half, GS)], ot[:])
        finally:
            for s in (s_w2, s_w1, s_x2T, s_qT, s_x1n, s_woh, s_hm, s_aoT,
                      s_pef, s_kv):
                s.close()


def kernel(**inputs):
    nc = build()
    x = np.ascontiguousarray(inputs["x"], dtype=np.float32)
    z = np.ascontiguousarray(inputs["z"], dtype=np.float32)
    base = {}
    for nm, _, _ in W2D + W2DR:
        base[nm] = np.ascontiguousarray(inputs[nm], dtype=np.float32)
    for nm, _ in W1D:
        base[nm] = np.ascontiguousarray(inputs[nm], dtype=np.float32)
    in_maps = []
    for c in range(B):
        m = dict(base)
        m["x"] = x[c]
        m["z"] = z[c:c + 1]
        in_maps.append(m)
    res = run_bass_kernel_spmd(nc, in_maps, list(range(B)))
    _cache["last"] = res
    return np.stack([res.results[c]["out"] for c in range(B)], axis=0)


# revision 15
# speedup vs baseline: 1.5864x; 1.0208x over previous
"""DiT block (Linformer attention + adaLN + MLP) on 8 TRN2 NeuronCores.

Sharding: data-parallel over batch (B=8 -> one batch element per core).

v2 layout (per core, S=2048 tokens, D=1024 features):
 - Phase A conditioning matvecs run in f32r (single-pass fp32 at full PE
   rate for moving dims >= 256); scale/offset rows are broadcast to 128
   partitions with gpsimd.partition_broadcast instead of PE ones-matmuls.
 - All other weights are converted once to f16 via DRAM->DRAM casting DMAs
   on the gpsimd (SWDGE) queue, then streamed as plain f16 HWDGE loads:
   no per-use casts, and the MLP weights are no longer re-streamed per
   token group (m1w resident in SBUF, m2w streamed f16 once per group).
 - Attention: per half-group (8 heads) all scores matmuls are issued as one
   64-row-tiled block, exps on scalar, then all attn@V matmuls; softmax
   denominators come from a fused ones-column appended to v_proj, and the
   1/denom normalization uses gpsimd partition_broadcast + vector
   reciprocal + psum multiply (no PE broadcast matmuls).
 - wo+residual is fused with adaLN2 + transposes per 128-token chunk, so
   attn_out is written to DRAM once and only re-read for the final
   residual add.

DMA queues: sync carries x/attn_sc/out streams and the f16 attention
weights; scalar carries phase-A f32r weights, EF/w1/w2 f16 streams and
bias rows; gpsimd (SWDGE) carries the 8 one-time casts plus all
latency-chained tiny transfers (h->cols, partition broadcasts) so they
never block a bulk queue at its head.
"""
import contextlib

import numpy as np

import concourse.bass as bass
import concourse.mybir as mybir
import concourse.tile as tile
from concourse import bacc
from concourse.bass import ds, ts
from concourse.bass_utils import run_bass_kernel_spmd
from concourse.masks import make_identity

f32 = mybir.dt.float32
f32r = mybir.dt.float32r
f16 = mybir.dt.float16
f8 = mybir.dt.float8e4
AF = mybir.ActivationFunctionType
OP = mybir.AluOpType

B, S, D, H, K, MLP, ZD = 8, 2048, 1024, 16, 256, 4096, 1024
DH = D // H      # 64
P = 128
SC = S // P      # 16 token chunks of 128
DC = D // P      # 8 feature chunks of 128
NG = 4           # token groups of 512
GS = 512
MC = MLP // P    # 32
KC = K // P      # 2
EPS = 1e-6

W2D = [("wq", D, D), ("wk", D, D), ("wv", D, D), ("wo", D, D),
       ("Ew", S, K), ("Fw", S, K),
       ("m1w", D, MLP), ("m2w", MLP, D)]
W2DR = [("h1w", ZD, D), ("g1w", D, D), ("be1w", D, D),
        ("h2w", ZD, D), ("g2w", D, D), ("be2w", D, D)]
W1D = [("bq", D), ("bk", D), ("bv", D), ("bo", D), ("Eb", K), ("Fb", K),
       ("h1b", D), ("g1b", D), ("be1b", D), ("h2b", D), ("g2b", D), ("be2b", D),
       ("m1b", MLP), ("m2b", D)]

_cache = {}


def build():
    if "nc" in _cache:
        return _cache["nc"]
    nc = bacc.Bacc("TRN2", target_bir_lowering=False, debug=False, num_devices=8)
    ap = {}
    ap["x"] = nc.dram_tensor("x", [S, D], f32, kind="ExternalInput").ap()
    ap["z"] = nc.dram_tensor("z", [1, ZD], f32r, kind="ExternalInput").ap()
    for nm, a, b in W2D:
        ap[nm] = nc.dram_tensor(nm, [a, b], f32, kind="ExternalInput").ap()
    for nm, a, b in W2DR:
        ap[nm] = nc.dram_tensor(nm, [a, b], f32r, kind="ExternalInput").ap()
    for nm, a in W1D:
        ap[nm] = nc.dram_tensor(nm, [a], f32, kind="ExternalInput").ap()
    out = nc.dram_tensor("out", [S, D], f32, kind="ExternalOutput").ap()
    with tile.TileContext(nc, trace_sim=False) as tc:
        _emit(nc, tc, ap, out)
    nc.compile()
    _cache["nc"] = nc
    return nc


def _emit(nc, tc, ap, out):
    ctx = contextlib.ExitStack()
    with ctx:
        # ---------- whole-kernel pools ----------
        const = ctx.enter_context(tc.tile_pool(name="const", bufs=1))
        cols = ctx.enter_context(tc.tile_pool(name="cols", bufs=1))
        bc = ctx.enter_context(tc.tile_pool(name="bc", bufs=1))
        dram = ctx.enter_context(tc.tile_pool(name="dram", bufs=1, space="DRAM"))

        attn_sc = [dram.tile([P, D], f32, tag=f"attn_sc{i}", name=f"attn_sc{i}")
                   for i in range(SC)]

        ident_f = const.tile([P, P], f32, tag="ident_f", name="ident_f")
        make_identity(nc, ident_f)
        ident_h = const.tile([P, P], f16, tag="ident_h", name="ident_h")
        nc.vector.tensor_copy(ident_h[:], ident_f[:])
        eps_t = const.tile([P, 1], f32, tag="eps", name="eps")
        nc.vector.memset(eps_t[:], EPS)
        ones_f = const.tile([P, 1], f32, tag="ones_f", name="ones_f")
        nc.vector.memset(ones_f[:], 1.0)
        onescol_h = const.tile([P, 1], f16, tag="onescol_h", name="onescol_h")
        nc.vector.tensor_copy(onescol_h[:], ones_f[:])
        ones_row = const.tile([1, P], f32, tag="ones_row", name="ones_row")
        nc.vector.memset(ones_row[:], 1.0)
        ones1_h = const.tile([1, P], f16, tag="ones1_h", name="ones1_h")
        nc.vector.tensor_copy(ones1_h[:], ones_row[:])

        def pe_bcast(psp, rowp, row_f32, out_t, n):
            """[1, n] f32 row -> [128, n] tile via PE ones-matmul."""
            row_h = rowp.tile([1, D], f16, tag="bc_row_h", name="bc_row_h",
                              bufs=2)
            nc.vector.tensor_copy(row_h[0:1, 0:n], row_f32[0:1, 0:n])
            for h in range(0, n, GS):
                w = min(GS, n - h)
                pt = psp.tile([P, GS], f32, tag="bc_ps", name="bc_ps", bufs=2)
                nc.tensor.matmul(pt[:, 0:w], ones1_h[:], row_h[0:1, h:h + w],
                                 start=True, stop=True)
                nc.scalar.copy(out_t[:, h:h + w], pt[:, 0:w])

        def col_load(name, n):
            """1-D DRAM vector [n*128] -> sbuf [128, n] (partition-major)."""
            t = cols.tile([P, n], f32, tag=f"cols_{name}", name=f"cols_{name}")
            for j in range(n):
                nc.scalar.dma_start(t[:, j:j + 1], ap[name][ds(P * j, P)])
            return t

        # broadcast result tiles (f16, whole-kernel)
        scale1_b = bc.tile([P, D], f16, tag="scale1_b", name="scale1_b")
        offset1_b = bc.tile([P, D], f16, tag="offset1_b", name="offset1_b")
        scale2_b = bc.tile([P, D], f16, tag="scale2_b", name="scale2_b")
        offset2_b = bc.tile([P, D], f16, tag="offset2_b", name="offset2_b")
        bo_b = bc.tile([P, D], f16, tag="bo_b", name="bo_b")
        m2b_b = bc.tile([P, D], f16, tag="m2b_b", name="m2b_b")

        zc_f = cols.tile([P, DC], f32r, tag="zc_f", name="zc_f")
        for j in range(DC):
            nc.scalar.dma_start(zc_f[:, j:j + 1], ap["z"][0:1, ds(P * j, P)])

        def vec_layer(vsb, vps, rowp, wname, lhs_cols, bias_row, act, out_row):
            """out_row[1, D] = act(lhs^T @ w + bias) with f32r weights."""
            pts = [vps.tile([1, GS], f32, tag=f"vps{h}", name=f"vps{h}")
                   for h in range(2)]
            for j in range(DC):
                wt = vsb.tile([P, D], f32r, tag=f"vw_{wname}", name=f"vw_{wname}",
                              bufs=4)
                nc.scalar.dma_start(wt[:], ap[wname][ds(P * j, P), :])
                for h in range(2):
                    nc.tensor.matmul(pts[h][:], lhs_cols[:, j:j + 1],
                                     wt[:, ds(GS * h, GS)],
                                     start=(j == 0), stop=(j == DC - 1))
            for h in range(2):
                pre = rowp.tile([1, GS], f32, tag=f"vpre{h}", name=f"vpre{h}",
                                bufs=2)
                nc.vector.tensor_add(pre[:], pts[h][:],
                                     bias_row[0:1, ds(GS * h, GS)])
                if act is None:
                    nc.vector.tensor_copy(out_row[0:1, ds(GS * h, GS)], pre[:])
                else:
                    nc.scalar.activation(out_row[0:1, ds(GS * h, GS)],
                                         pre[:], act)

        def a_chain(vsb, vps, rowp, hw, hb, gw, gb, bw, bb, tagn, sc_b, of_b):
            def row_load(name):
                t = rowp.tile([1, D], f32, tag="arow_b", name=f"row_{name}",
                              bufs=2)
                nc.scalar.dma_start(t[:], ap[name][0:D])
                return t

            hb_row = row_load(hb)
            h_row = rowp.tile([1, D], f32, tag="h_row", name=f"h_{tagn}")
            vec_layer(vsb, vps, rowp, hw, zc_f, hb_row, AF.Silu, h_row)
            h_row_r = rowp.tile([1, D], f32r, tag="h_row_r", name=f"hr_{tagn}")
            nc.vector.tensor_copy(h_row_r[:], h_row[:])
            h_c = cols.tile([P, DC], f32r, tag=f"c_{tagn}", name=f"c_{tagn}")
            for j in range(DC):
                nc.gpsimd.dma_start(h_c[:, j:j + 1], h_row_r[0:1, ds(P * j, P)])
            gb_row = row_load(gb)
            sc_row = rowp.tile([1, D], f32, tag="sc_row", name=f"sc_{tagn}")
            vec_layer(vsb, vps, rowp, gw, h_c, gb_row, None, sc_row)
            pe_bcast(vps, rowp, sc_row, sc_b, D)
            bb_row = row_load(bb)
            of_row = rowp.tile([1, D], f32, tag="of_row", name=f"of_{tagn}")
            vec_layer(vsb, vps, rowp, bw, h_c, bb_row, None, of_row)
            pe_bcast(vps, rowp, of_row, of_b, D)

        # =========== phase A1: adaLN1 conditioning vectors (f32r) ===========
        with (
            tc.tile_pool(name="vec1_sb", bufs=1) as vsb1,
            tc.tile_pool(name="row1_sb", bufs=1) as rowp1,
            tc.tile_pool(name="vec1_ps", bufs=2, space="PSUM") as vps1,
        ):
            a_chain(vsb1, vps1, rowp1, "h1w", "h1b", "g1w", "g1b",
                    "be1w", "be1b", "h1", scale1_b, offset1_b)
        # manual pool stacks (LIFO per SBUF side)
        s_woh = contextlib.ExitStack()   # left (created first: popped last)
        s_qT = contextlib.ExitStack()    # left
        s_x1n = contextlib.ExitStack()   # left
        s_x2T = contextlib.ExitStack()   # left
        s_w1 = contextlib.ExitStack()    # left
        s_w2 = contextlib.ExitStack()    # left (E only)
        s_kv = contextlib.ExitStack()    # right (kpT/vpe, lingers under aoT)
        s_pef = contextlib.ExitStack()   # right (pefE/F + colsums, popped at KV end)
        s_aoT = contextlib.ExitStack()   # right
        s_hm = contextlib.ExitStack()    # right
        try:
            # ===== phase B: adaLN1 + transposes + qT =====
            qT_p = s_qT.enter_context(tc.tile_pool(name="qT", bufs=1))
            qT = [[qT_p.tile([P, GS], f16, tag=f"qT_{j}_{g}", name=f"qT_{j}_{g}")
                   for g in range(NG)] for j in range(DC)]
            x1n_p = s_x1n.enter_context(tc.tile_pool(name="x1nat", bufs=1))
            x1n = []
            with (
                tc.tile_pool(name="wq_sb", bufs=1) as wqsb,
                tc.tile_pool(name="ln1_sb", bufs=2) as ln_sb,
                tc.tile_pool(name="x1Trot", bufs=1) as x1t_p,
                tc.tile_pool(name="tp1_ps", bufs=2, space="PSUM") as ln_ps,
                tc.tile_pool(name="q_ps", bufs=3, space="PSUM") as qps,
            ):
                wq_r = []
                for j in range(DC):
                    wf = wqsb.tile([P, D], f32, tag="wq_f", name="wq_f",
                                   bufs=2)
                    nc.scalar.dma_start(wf[:], ap["wq"][ds(P * j, P), :])
                    wr = wqsb.tile([P, D], f16, tag=f"wq_r{j}", name=f"wq_r{j}")
                    nc.scalar.copy(wr[:], wf[:])
                    wq_r.append(wr)
                bq_c = col_load("bq", DC)
                bk_c = col_load("bk", DC)
                Fb_c = col_load("Fb", KC)
                for g in range(NG):
                    x1T_g = [x1t_p.tile([P, GS], f16, tag=f"x1T_{j}",
                                        name=f"x1T_{j}") for j in range(DC)]
                    for ii in range(4):
                        i = 4 * g + ii
                        xt = ln_sb.tile([P, D], f32, tag="ln_in",
                                        name=f"ln_in{i}", bufs=4)
                        nc.sync.dma_start(xt[:], ap["x"][ds(P * i, P), :])
                        st = ln_sb.tile([P, 2, 6], f32, tag="ln_st",
                                        name="ln_st")
                        nc.vector.bn_stats(st[:, 0, :], xt[:, 0:GS])
                        nc.vector.bn_stats(st[:, 1, :], xt[:, GS:D])
                        mv = ln_sb.tile([P, 2], f32, tag="ln_mv", name="ln_mv")
                        nc.vector.bn_aggr(mv[:], st[:])
                        sd = ln_sb.tile([P, 1], f32, tag="ln_sd",
                                        name="ln_sd")
                        nc.scalar.activation(sd[:], mv[:, 1:2], AF.Sqrt,
                                             bias=eps_t[:])
                        rstd = ln_sb.tile([P, 1], f32, tag="ln_rstd",
                                          name="ln_rstd")
                        nc.vector.reciprocal_approx_fast(rstd[:], sd[:])
                        nmr = ln_sb.tile([P, 1], f32, tag="ln_nmr",
                                         name="ln_nmr")
                        nc.vector.tensor_scalar(nmr[:], mv[:, 0:1], rstd[:],
                                                -1.0, OP.mult, OP.mult)
                        xn = ln_sb.tile([P, D], f32, tag="ln_xn", name="ln_xn")
                        nc.scalar.activation(xn[:], xt[:], AF.Identity,
                                             bias=nmr[:], scale=rstd[:])
                        nc.vector.tensor_mul(xn[:], xn[:], scale1_b[:])
                        x1t = x1n_p.tile([P, D], f16, tag=f"nat{i}",
                                         name=f"nat{i}")
                        nc.vector.tensor_add(x1t[:], xn[:], offset1_b[:])
                        x1n.append(x1t)
                        for j in range(DC):
                            pt = ln_ps.tile([P, P], f16, tag="tp_ps",
                                            name="tp_ps")
                            nc.tensor.transpose(pt[:], x1t[:, ds(P * j, P)],
                                                ident_h[:])
                            nc.scalar.copy(
                                x1T_g[j][:, ds(P * ii, P)], pt[:])
                    for jo in range(DC):
                        pt = qps.tile([P, GS], f32, tag="q_ps", name="q_ps")
                        for j in range(DC):
                            nc.tensor.matmul(pt[:],
                                             wq_r[j][:, ds(P * jo, P)],
                                             x1T_g[j][:],
                                             start=(j == 0),
                                             stop=(j == DC - 1))
                        nc.scalar.activation(qT[jo][g][:], pt[:], AF.Identity,
                                             bias=bq_c[:, jo:jo + 1])

            # ===== phase B2: P_EF = x1^T @ [Ew|Fw], colsums fused after =====
            kv_sb = s_kv.enter_context(
                tc.tile_pool(name="kv_sb", bufs=1, side="right"))
            pef_sb = s_pef.enter_context(
                tc.tile_pool(name="pef_sb", bufs=1, side="right"))
            pefE = [pef_sb.tile([P, K], f16, tag=f"pefE{j}", name=f"pefE{j}")
                    for j in range(DC)]
            pefF = [pef_sb.tile([P, K], f16, tag=f"pefF{j}", name=f"pefF{j}")
                    for j in range(DC)]
            cs_row = pef_sb.tile([1, 2 * K], f32, tag="cs", name="cs")
            with tc.tile_pool(name="ef_sb", bufs=1) as efsb:
                ef_h = []
                for i in range(SC):
                    ff = efsb.tile([P, 2 * K], f32, tag="ef_f", name="ef_f",
                                   bufs=3)
                    nc.scalar.dma_start(ff[:, 0:K], ap["Ew"][ds(P * i, P), :])
                    nc.scalar.dma_start(ff[:, K:2 * K],
                                        ap["Fw"][ds(P * i, P), :])
                    t = efsb.tile([P, 2 * K], f16, tag=f"ef{i}", name=f"ef{i}")
                    nc.scalar.copy(t[:], ff[:])
                    ef_h.append(t)
                m1b_c = col_load("m1b", MC)
                with tc.tile_pool(name="pef_ps", bufs=1, space="PSUM") as pfps:
                    pef_ps = [pfps.tile([P, 2 * K], f32, tag=f"pefp{j}",
                                        name=f"pefp{j}") for j in range(DC)]
                    for i in range(SC):
                        for j in range(DC):
                            nc.tensor.matmul(pef_ps[j][:],
                                             x1n[i][:, ds(P * j, P)],
                                             ef_h[i][:],
                                             start=(i == 0), stop=(i == SC - 1))
                    for j in range(DC):
                        nc.scalar.copy(pefE[j][:], pef_ps[j][:, 0:K])
                        nc.scalar.copy(pefF[j][:], pef_ps[j][:, K:2 * K])
                with tc.tile_pool(name="cs_ps", bufs=1, space="PSUM") as csps:
                    cs_ps = csps.tile([1, 2 * K], f32, tag="cs_ps",
                                      name="cs_ps")
                    for i in range(SC):
                        nc.tensor.matmul(cs_ps[:], onescol_h[:], ef_h[i][:],
                                         start=(i == 0), stop=(i == SC - 1))
                    nc.vector.tensor_copy(cs_row[:], cs_ps[:])
            s_x1n.close()

            # ===== phase KV: k_projT, v_proj_ext =====
            kpT = [kv_sb.tile([P, K], f16, tag=f"kpT{j}", name=f"kpT{j}")
                   for j in range(DC)]
            vpe = [kv_sb.tile([P, 65 * H], f16, tag=f"vpe{c}", name=f"vpe{c}")
                   for c in range(KC)]
            with (
                tc.tile_pool(name="kv_w", bufs=1) as kvw,
                tc.tile_pool(name="kv_bias", bufs=1) as kvb,
                tc.tile_pool(name="kv_tmp", bufs=2) as kvt,
                tc.tile_pool(name="kv_ps", bufs=2, space="PSUM") as kvps,
            ):
                Eb_row = kvb.tile([1, K], f32, tag="Eb_row", name="Eb_row")
                nc.scalar.dma_start(Eb_row[:], ap["Eb"][0:K])
                Eb_b = kvb.tile([P, K], f32, tag="Eb_b", name="Eb_b")
                pe_bcast(kvps, kvt, Eb_row, Eb_b, K)
                csE_b = kvb.tile([P, K], f32, tag="csE_b", name="csE_b")
                pe_bcast(kvps, kvt, cs_row, csE_b, K)
                bv_row = kvb.tile([1, D], f32, tag="bv_row", name="bv_row")
                nc.scalar.dma_start(bv_row[:], ap["bv"][0:D])
                bv_b = kvb.tile([P, D], f32, tag="bv_b", name="bv_b")
                pe_bcast(kvps, kvt, bv_row, bv_b, D)
                csF_c = kvb.tile([P, KC], f32, tag="csF_c", name="csF_c")
                for c in range(KC):
                    nc.gpsimd.dma_start(csF_c[:, c:c + 1],
                                        cs_row[0:1, ds(K + P * c, P)])
                kp_bias = []
                for j in range(DC):
                    bt = kvb.tile([P, K], f32, tag=f"kpb{j}", name=f"kpb{j}")
                    nc.vector.tensor_scalar(bt[:], csE_b[:], bk_c[:, j:j + 1],
                                            None, OP.mult)
                    nc.vector.tensor_add(bt[:], bt[:], Eb_b[:])
                    kp_bias.append(bt)
                vp_bias = []
                for c in range(KC):
                    bt = kvb.tile([P, D], f32, tag=f"vpb{c}", name=f"vpb{c}")
                    nc.vector.tensor_scalar(bt[:], bv_b[:], csF_c[:, c:c + 1],
                                            Fb_c[:, c:c + 1], OP.mult, OP.add)
                    vp_bias.append(bt)

                wk_r, wv_r = [], []
                for j in range(DC):
                    for nm, lst, tg in (("wk", wk_r, "wk"), ("wv", wv_r, "wv")):
                        wf = kvw.tile([P, D], f32, tag=f"{tg}_f",
                                      name=f"{tg}_f", bufs=2)
                        nc.sync.dma_start(wf[:], ap[nm][ds(P * j, P), :])
                        wr = kvw.tile([P, D], f16, tag=f"{tg}_r{j}",
                                      name=f"{tg}_r{j}")
                        nc.scalar.copy(wr[:], wf[:])
                        lst.append(wr)
                for jo in range(DC):
                    pt = kvps.tile([P, K], f32, tag="kp_ps", name="kp_ps")
                    for j in range(DC):
                        nc.tensor.matmul(pt[:], wk_r[j][:, ds(P * jo, P)],
                                         pefE[j][:],
                                         start=(j == 0), stop=(j == DC - 1))
                    nc.vector.tensor_add(kpT[jo][:], pt[:], kp_bias[jo][:])
                for hf in range(2):
                    for c in range(KC):
                        pt = kvps.tile([P, GS], f32, tag="vp_ps", name="vp_ps")
                        for j in range(DC):
                            nc.tensor.matmul(pt[:], pefF[j][:, ds(P * c, P)],
                                             wv_r[j][:, ds(GS * hf, GS)],
                                             start=(j == 0), stop=(j == DC - 1))
                        tmp = kvt.tile([P, GS], f32, tag="vp_tmp",
                                       name="vp_tmp")
                        nc.vector.tensor_add(tmp[:], pt[:],
                                             vp_bias[c][:, ds(GS * hf, GS)])
                        for hh in range(8):
                            h = 8 * hf + hh
                            nc.vector.tensor_copy(vpe[c][:, ds(65 * h, 64)],
                                                  tmp[:, ds(64 * hh, 64)])
                for c in range(KC):
                    for h in range(H):
                        nc.vector.tensor_copy(vpe[c][:, ds(65 * h + 64, 1)],
                                              ones_f[:, 0:1])

            s_pef.close()

            # ===== phase A2: adaLN2 conditioning vectors =====
            with (
                tc.tile_pool(name="vec2_sb", bufs=1) as vsb2,
                tc.tile_pool(name="row2_sb", bufs=1) as rowp2,
                tc.tile_pool(name="vec2_ps", bufs=2, space="PSUM") as vps2,
            ):
                a_chain(vsb2, vps2, rowp2, "h2w", "h2b", "g2w", "g2b",
                        "be2w", "be2b", "h2", scale2_b, offset2_b)
                brow = rowp2.tile([1, D], f32, tag="brow", name="bo_row",
                                  bufs=2)
                nc.scalar.dma_start(brow[:], ap["bo"][0:D])
                pe_bcast(vps2, rowp2, brow, bo_b, D)
                brow2 = rowp2.tile([1, D], f32, tag="brow", name="m2b_row",
                                   bufs=2)
                nc.scalar.dma_start(brow2[:], ap["m2b"][0:D])
                pe_bcast(vps2, rowp2, brow2, m2b_b, D)

            # ===== load wo (f16) for C3 =====
            woh_sb = s_woh.enter_context(
                tc.tile_pool(name="woh_sb", bufs=1, side="right"))
            wo_r = []
            for j in range(DC):
                wf = woh_sb.tile([P, D], f32, tag="wo_f", name="wo_f", bufs=2)
                nc.sync.dma_start(wf[:], ap["wo"][ds(P * j, P), :])
                wr = woh_sb.tile([P, D], f16, tag=f"wo_r{j}", name=f"wo_r{j}")
                nc.vector.tensor_copy(wr[:], wf[:])
                wo_r.append(wr)

            # ===== phase C2: attention =====
            aoT_p = s_aoT.enter_context(
                tc.tile_pool(name="aoT", bufs=1, side="right"))
            aoT = [[aoT_p.tile([P, GS], f16, tag=f"aoT_{j}_{g}",
                               name=f"aoT_{j}_{g}")
                    for g in range(NG)] for j in range(DC)]
            with (
                tc.tile_pool(name="exp_sb", bufs=1) as expsb,
                tc.tile_pool(name="nrm_sb", bufs=1) as nrmsb,
                tc.tile_pool(name="sc_ps", bufs=4, space="PSUM") as scps,
                tc.tile_pool(name="av_ps", bufs=2, space="PSUM") as avps,
                tc.tile_pool(name="bc2_ps", bufs=2, space="PSUM") as bcps,
            ):
                def emit_scores_half(hg):
                    """scores+exp for the 8 heads of half-group hg (0..7)."""
                    g, hb = hg // 2, (hg % 2) * 8
                    exps = []
                    for ph in range(4):
                        for e in range(2):
                            h = hb + 2 * ph + e
                            j, r0 = h // 2, 64 * (h % 2)
                            ets = []
                            for c in range(KC):
                                spt = scps.tile([P, GS], f32, tag="sc",
                                                name="sc")
                                nc.tensor.matmul(
                                    spt[:],
                                    kpT[j][r0:r0 + 64, ds(P * c, P)],
                                    qT[j][g][r0:r0 + 64, :],
                                    start=True, stop=True)
                                et = expsb.tile([P, GS], f16, tag="exp",
                                                name="exp", bufs=34)
                                nc.scalar.activation(et[:], spt[:], AF.Exp,
                                                     scale=0.125)
                                ets.append(et)
                            exps.append((h, ets))
                    return exps

                def emit_av_half(exps, g):
                    items = []
                    for h, ets in exps:
                        j, r0 = h // 2, 64 * (h % 2)
                        apt = avps.tile([65, GS], f32, tag="av", name="av")
                        for c in range(KC):
                            nc.tensor.matmul(apt[:], vpe[c][:, ds(65 * h, 65)],
                                             ets[c][:],
                                             start=(c == 0), stop=(c == KC - 1))
                        den = nrmsb.tile([1, GS], f16, tag="den", name="den",
                                         bufs=10)
                        nc.vector.tensor_copy(den[:], apt[64:65, :])
                        to = nrmsb.tile([64, GS], f16, tag="to", name="to",
                                        bufs=10)
                        nc.vector.tensor_copy(to[:], apt[0:64, :])
                        items.append((h, den, to))
                    # batched denominator broadcast (one PE mode switch)
                    for h, den, to in items:
                        j, r0 = h // 2, 64 * (h % 2)
                        bpt = bcps.tile([64, GS], f32, tag="bc2", name="bc2")
                        nc.tensor.matmul(bpt[:], ones1_h[0:1, 0:64], den[:],
                                         start=True, stop=True)
                        rec = nrmsb.tile([64, GS], f32, tag="rec", name="rec",
                                         bufs=3)
                        nc.vector.reciprocal_approx_fast(rec[:], bpt[:])
                        nc.vector.tensor_mul(aoT[j][g][r0:r0 + 64, :],
                                             to[:], rec[:])

                prev = None
                for hg in range(2 * NG):
                    cur = emit_scores_half(hg)
                    if prev is not None:
                        emit_av_half(*prev)
                    prev = (cur, hg // 2)
                emit_av_half(*prev)
            s_qT.close()

            # ===== phase C3: wo + residual + adaLN2 + transposes, fused =====
            x2T_p = s_x2T.enter_context(tc.tile_pool(name="x2T", bufs=1))
            x2T = [x2T_p.tile([P, S], f16, tag=f"x2T_{j}", name=f"x2T_{j}")
                   for j in range(DC)]
            with (
                tc.tile_pool(name="c3_sb", bufs=1) as c3sb,
                tc.tile_pool(name="wo_ps", bufs=3, space="PSUM") as wops,
                tc.tile_pool(name="tp2_ps", bufs=2, space="PSUM") as tp2ps,
            ):
                pend = []
                for i in range(SC):
                    g, c = i // 4, (i % 4) * P
                    xt = c3sb.tile([P, D], f32, tag="res_x", name="res_x",
                                   bufs=2)
                    nc.sync.dma_start(xt[:], ap["x"][ds(P * i, P), :])
                    at = c3sb.tile([P, D], f32, tag="attn_nat",
                                   name="attn_nat", bufs=2)
                    for hf in range(2):
                        pt = wops.tile([P, GS], f32, tag="wo_ps", name="wo_ps")
                        for j in range(DC):
                            nc.tensor.matmul(pt[:], aoT[j][g][:, ds(c, P)],
                                             wo_r[j][:, ds(GS * hf, GS)],
                                             start=(j == 0), stop=(j == DC - 1))
                        tm = c3sb.tile([P, GS], f32, tag="wo_tmp",
                                       name="wo_tmp", bufs=2)
                        nc.vector.tensor_add(tm[:], pt[:],
                                             bo_b[:, ds(GS * hf, GS)])
                        nc.vector.tensor_add(at[:, ds(GS * hf, GS)], tm[:],
                                             xt[:, ds(GS * hf, GS)])
                    nc.sync.dma_start(attn_sc[i][:], at[:])
                    st = c3sb.tile([P, 2, 6], f32, tag="ln2_st",
                                   name="ln2_st", bufs=2)
                    nc.vector.bn_stats(st[:, 0, :], at[:, 0:GS])
                    nc.vector.bn_stats(st[:, 1, :], at[:, GS:D])
                    mv = c3sb.tile([P, 2], f32, tag="ln2_mv", name="ln2_mv")
                    nc.vector.bn_aggr(mv[:], st[:])
                    sd = c3sb.tile([P, 1], f32, tag="ln2_sd",
                                   name="ln2_sd")
                    nc.scalar.activation(sd[:], mv[:, 1:2], AF.Sqrt,
                                         bias=eps_t[:])
                    rstd = c3sb.tile([P, 1], f32, tag="ln2_rstd",
                                     name="ln2_rstd")
                    nc.vector.reciprocal_approx_fast(rstd[:], sd[:])
                    nmr = c3sb.tile([P, 1], f32, tag="ln2_nmr", name="ln2_nmr")
                    nc.vector.tensor_scalar(nmr[:], mv[:, 0:1], rstd[:],
                                            -1.0, OP.mult, OP.mult)
                    xn = c3sb.tile([P, D], f32, tag="ln2_xn", name="ln2_xn")
                    nc.scalar.activation(xn[:], at[:], AF.Identity,
                                         bias=nmr[:], scale=rstd[:])
                    nc.vector.tensor_mul(xn[:], xn[:], scale2_b[:])
                    x2t = c3sb.tile([P, D], f16, tag="x2nat", name="x2nat",
                                    bufs=3)
                    nc.vector.tensor_add(x2t[:], xn[:], offset2_b[:])
                    pend.append((i, x2t))
                    if len(pend) > 1:
                        pi, px = pend.pop(0)
                        for j in range(DC):
                            pt = tp2ps.tile([P, P], f16, tag="tp2_ps",
                                            name="tp2_ps")
                            nc.tensor.transpose(pt[:], px[:, ds(P * j, P)],
                                                ident_h[:])
                            nc.scalar.copy(x2T[j][:, ds(P * pi, P)], pt[:])
                for pi, px in pend:
                    for j in range(DC):
                        pt = tp2ps.tile([P, P], f16, tag="tp2_ps",
                                        name="tp2_ps")
                        nc.tensor.transpose(pt[:], px[:, ds(P * j, P)],
                                            ident_h[:])
                        nc.scalar.copy(x2T[j][:, ds(P * pi, P)], pt[:])
            s_aoT.close()
            s_woh.close()
            s_kv.close()

            # ===== phase E: MLP per token group =====
            hm_p = s_hm.enter_context(
                tc.tile_pool(name="hm", bufs=1, side="right"))
            hm = [hm_p.tile([P, GS], f16, tag=f"hm{m}", name=f"hm{m}")
                  for m in range(MC)]
            # w2 resident f16, filled during m1(g0) (f32 stream + DVE cast)
            w2sb_p = s_w2.enter_context(tc.tile_pool(name="w2sb", bufs=1))
            w2sb = [None] * MC

            def w2_fill(ms):
                for m in ms:
                    wf = w2sb_p.tile([P, D], f32, tag="w2_f", name="w2_f",
                                     bufs=3)
                    nc.scalar.dma_start(wf[:], ap["m2w"][ds(P * m, P), :])
                    t = w2sb_p.tile([P, D], f16, tag=f"w2_{m}", name=f"w2_{m}")
                    nc.vector.tensor_copy(t[:], wf[:])
                    w2sb[m] = t
            # w1 streamed f32 per (group, m-quarter, j) + inline gpsimd cast;
            # m1 runs in m-quarter passes so psum for m1 and m2 share 8 banks
            QW = MLP // 4  # 1024 m-columns per quarter
            with (
                tc.tile_pool(name="w1st", bufs=1) as w1st,
                tc.tile_pool(name="e_sb", bufs=2) as esb,
                tc.tile_pool(name="mlp_ps", bufs=1, space="PSUM") as mlpps,
            ):
                for g in range(NG):
                    for q in range(4):
                        pts = [mlpps.tile([P, GS], f32, tag=f"p{mm}",
                                          name=f"p{mm}") for mm in range(8)]
                        for j in range(DC):
                            wf = w1st.tile([P, QW], f32, tag="w1_f",
                                           name="w1_f", bufs=2)
                            nc.sync.dma_start(
                                wf[:], ap["m1w"][ds(P * j, P), ds(QW * q, QW)])
                            wt = w1st.tile([P, QW], f16, tag="w1_h",
                                           name="w1_h", bufs=2)
                            nc.vector.tensor_copy(wt[:], wf[:])
                            for mm in range(8):
                                nc.tensor.matmul(
                                    pts[mm][:], wt[:, ds(P * mm, P)],
                                    x2T[j][:, ds(GS * g, GS)],
                                    start=(j == 0), stop=(j == DC - 1))
                        for mm in range(8):
                            m = 8 * q + mm
                            nc.scalar.activation(hm[m][:], pts[mm][:], AF.Gelu,
                                                 bias=m1b_c[:, m:m + 1])
                        if g == 0:
                            w2_fill(range(8 * q, 8 * q + 8))
                    for half in range(2):
                        m2p = [mlpps.tile([P, GS], f32, tag=f"p{ss}",
                                          name=f"m2p{ss}") for ss in range(4)]
                        for m in range(MC):
                            for ss in range(4):
                                nc.tensor.matmul(
                                    m2p[ss][:], hm[m][:, ds(P * ss, P)],
                                    w2sb[m][:, ds(GS * half, GS)],
                                    start=(m == 0), stop=(m == MC - 1))
                        for ss in range(4):
                            i = 4 * g + ss
                            rt = esb.tile([P, GS], f32, tag="res_a",
                                          name="res_a")
                            nc.sync.dma_start(
                                rt[:], attn_sc[i][:, ds(GS * half, GS)])
                            tm = esb.tile([P, GS], f32, tag="e_tmp",
                                          name="e_tmp")
                            nc.vector.tensor_add(tm[:], m2p[ss][:],
                                                 m2b_b[:, ds(GS * half, GS)])
                            ot = esb.tile([P, GS], f32, tag="e_out",
                                          name="e_out")
                            nc.vector.tensor_add(ot[:], tm[:], rt[:])
                            nc.sync.dma_start(
                                out[ds(P * i, P), ds(GS * half, GS)], ot[:])
        finally:
            for s in (s_w2, s_w1, s_x2T, s_qT, s_x1n, s_woh, s_hm, s_aoT,
                      s_pef, s_kv):
                s.close()


def kernel(**inputs):
    nc = build()
    x = np.ascontiguousarray(inputs["x"], dtype=np.float32)
    z = np.ascontiguousarray(inputs["z"], dtype=np.float32)
    base = {}
    for nm, _, _ in W2D + W2DR:
        base[nm] = np.ascontiguousarray(inputs[nm], dtype=np.float32)
    for nm, _ in W1D:
        base[nm] = np.ascontiguousarray(inputs[nm], dtype=np.float32)
    in_maps = []
    for c in range(B):
        m = dict(base)
        m["x"] = x[c]
        m["z"] = z[c:c + 1]
        in_maps.append(m)
    res = run_bass_kernel_spmd(nc, in_maps, list(range(B)))
    _cache["last"] = res
    return np.stack([res.results[c]["out"] for c in range(B)], axis=0)


# revision 18
# speedup vs baseline: 1.8997x; 1.1975x over previous
"""DiT block (Linformer attention + adaLN + MLP) on 8 TRN2 NeuronCores.

Sharding: data-parallel over batch (B=8 -> one batch element per core).

v2 layout (per core, S=2048 tokens, D=1024 features):
 - Phase A conditioning matvecs run in f32r (single-pass fp32 at full PE
   rate for moving dims >= 256); scale/offset rows are broadcast to 128
   partitions with gpsimd.partition_broadcast instead of PE ones-matmuls.
 - All other weights are converted once to f16 via DRAM->DRAM casting DMAs
   on the gpsimd (SWDGE) queue, then streamed as plain f16 HWDGE loads:
   no per-use casts, and the MLP weights are no longer re-streamed per
   token group (m1w resident in SBUF, m2w streamed f16 once per group).
 - Attention: per half-group (8 heads) all scores matmuls are issued as one
   64-row-tiled block, exps on scalar, then all attn@V matmuls; softmax
   denominators come from a fused ones-column appended to v_proj, and the
   1/denom normalization uses gpsimd partition_broadcast + vector
   reciprocal + psum multiply (no PE broadcast matmuls).
 - wo+residual is fused with adaLN2 + transposes per 128-token chunk, so
   attn_out is written to DRAM once and only re-read for the final
   residual add.

DMA queues: sync carries x/attn_sc/out streams and the f16 attention
weights; scalar carries phase-A f32r weights, EF/w1/w2 f16 streams and
bias rows; gpsimd (SWDGE) carries the 8 one-time casts plus all
latency-chained tiny transfers (h->cols, partition broadcasts) so they
never block a bulk queue at its head.
"""
import contextlib

import numpy as np

import concourse.bass as bass
import concourse.mybir as mybir
import concourse.tile as tile
from concourse import bacc
from concourse.bass import ds, ts
from concourse.bass_utils import run_bass_kernel_spmd
from concourse.masks import make_identity

f32 = mybir.dt.float32
f32r = mybir.dt.float32r
f16 = mybir.dt.float16
f8 = mybir.dt.float8e4
AF = mybir.ActivationFunctionType
OP = mybir.AluOpType

B, S, D, H, K, MLP, ZD = 8, 2048, 1024, 16, 256, 4096, 1024
DH = D // H      # 64
P = 128
SC = S // P      # 16 token chunks of 128
DC = D // P      # 8 feature chunks of 128
NG = 4           # token groups of 512
GS = 512
MC = MLP // P    # 32
KC = K // P      # 2
EPS = 1e-6

W2D = [("wq", D, D), ("wk", D, D), ("wv", D, D), ("wo", D, D),
       ("Ew", S, K), ("Fw", S, K),
       ("m1w", D, MLP), ("m2w", MLP, D)]
W2DR = [("h1w", ZD, D), ("g1w", D, D), ("be1w", D, D),
        ("h2w", ZD, D), ("g2w", D, D), ("be2w", D, D)]
W1D = [("bq", D), ("bk", D), ("bv", D), ("bo", D), ("Eb", K), ("Fb", K),
       ("h1b", D), ("g1b", D), ("be1b", D), ("h2b", D), ("g2b", D), ("be2b", D),
       ("m1b", MLP), ("m2b", D)]

_cache = {}


def build():
    if "nc" in _cache:
        return _cache["nc"]
    nc = bacc.Bacc("TRN2", target_bir_lowering=False, debug=False, num_devices=8)
    ap = {}
    ap["x"] = nc.dram_tensor("x", [S, D], f32, kind="ExternalInput").ap()
    ap["z"] = nc.dram_tensor("z", [1, ZD], f32r, kind="ExternalInput").ap()
    for nm, a, b in W2D:
        ap[nm] = nc.dram_tensor(nm, [a, b], f32, kind="ExternalInput").ap()
    for nm, a, b in W2DR:
        ap[nm] = nc.dram_tensor(nm, [a, b], f32r, kind="ExternalInput").ap()
    for nm, a in W1D:
        ap[nm] = nc.dram_tensor(nm, [a], f32, kind="ExternalInput").ap()
    out = nc.dram_tensor("out", [S, D], f32, kind="ExternalOutput").ap()
    with tile.TileContext(nc, trace_sim=False) as tc:
        _emit(nc, tc, ap, out)
    nc.compile()
    _cache["nc"] = nc
    return nc


def _emit(nc, tc, ap, out):
    ctx = contextlib.ExitStack()
    with ctx:
        # ---------- whole-kernel pools ----------
        const = ctx.enter_context(tc.tile_pool(name="const", bufs=1))
        cols = ctx.enter_context(tc.tile_pool(name="cols", bufs=1))
        bc = ctx.enter_context(tc.tile_pool(name="bc", bufs=1))
        dram = ctx.enter_context(tc.tile_pool(name="dram", bufs=1, space="DRAM"))

        attn_sc = [dram.tile([P, D], f32, tag=f"attn_sc{i}", name=f"attn_sc{i}")
                   for i in range(SC)]

        ident_f = const.tile([P, P], f32, tag="ident_f", name="ident_f")
        make_identity(nc, ident_f)
        ident_h = const.tile([P, P], f16, tag="ident_h", name="ident_h")
        nc.vector.tensor_copy(ident_h[:], ident_f[:])
        eps_t = const.tile([P, 1], f32, tag="eps", name="eps")
        nc.vector.memset(eps_t[:], EPS)
        ones_f = const.tile([P, 1], f32, tag="ones_f", name="ones_f")
        nc.vector.memset(ones_f[:], 1.0)
        onescol_h = const.tile([P, 1], f16, tag="onescol_h", name="onescol_h")
        nc.vector.tensor_copy(onescol_h[:], ones_f[:])
        ones_row = const.tile([1, P], f32, tag="ones_row", name="ones_row")
        nc.vector.memset(ones_row[:], 1.0)
        ones1_h = const.tile([1, P], f16, tag="ones1_h", name="ones1_h")
        nc.vector.tensor_copy(ones1_h[:], ones_row[:])

        def pe_bcast(psp, rowp, row_f32, out_t, n):
            """[1, n] f32 row -> [128, n] tile via PE ones-matmul."""
            row_h = rowp.tile([1, D], f16, tag="bc_row_h", name="bc_row_h",
                              bufs=2)
            nc.vector.tensor_copy(row_h[0:1, 0:n], row_f32[0:1, 0:n])
            for h in range(0, n, GS):
                w = min(GS, n - h)
                pt = psp.tile([P, GS], f32, tag="bc_ps", name="bc_ps", bufs=2)
                nc.tensor.matmul(pt[:, 0:w], ones1_h[:], row_h[0:1, h:h + w],
                                 start=True, stop=True)
                nc.scalar.copy(out_t[:, h:h + w], pt[:, 0:w])

        def col_load(name, n):
            """1-D DRAM vector [n*128] -> sbuf [128, n] (partition-major)."""
            t = cols.tile([P, n], f32, tag=f"cols_{name}", name=f"cols_{name}")
            for j in range(n):
                nc.scalar.dma_start(t[:, j:j + 1], ap[name][ds(P * j, P)])
            return t

        # broadcast result tiles (f16, whole-kernel)
        scale1_b = bc.tile([P, D], f16, tag="scale1_b", name="scale1_b")
        offset1_b = bc.tile([P, D], f16, tag="offset1_b", name="offset1_b")
        scale2_b = bc.tile([P, D], f16, tag="scale2_b", name="scale2_b")
        offset2_b = bc.tile([P, D], f16, tag="offset2_b", name="offset2_b")
        bo_b = bc.tile([P, D], f16, tag="bo_b", name="bo_b")
        m2b_b = bc.tile([P, D], f16, tag="m2b_b", name="m2b_b")

        zc_f = cols.tile([P, DC], f32r, tag="zc_f", name="zc_f")
        for j in range(DC):
            nc.scalar.dma_start(zc_f[:, j:j + 1], ap["z"][0:1, ds(P * j, P)])

        def vec_layer(vsb, vps, rowp, wname, lhs_cols, bias_row, act, out_row):
            """out_row[1, D] = act(lhs^T @ w + bias) with f32r weights."""
            pts = [vps.tile([1, GS], f32, tag=f"vps{h}", name=f"vps{h}")
                   for h in range(2)]
            for j in range(DC):
                wt = vsb.tile([P, D], f32r, tag=f"vw_{wname}", name=f"vw_{wname}",
                              bufs=4)
                nc.scalar.dma_start(wt[:], ap[wname][ds(P * j, P), :])
                for h in range(2):
                    nc.tensor.matmul(pts[h][:], lhs_cols[:, j:j + 1],
                                     wt[:, ds(GS * h, GS)],
                                     start=(j == 0), stop=(j == DC - 1))
            for h in range(2):
                pre = rowp.tile([1, GS], f32, tag=f"vpre{h}", name=f"vpre{h}",
                                bufs=2)
                nc.vector.tensor_add(pre[:], pts[h][:],
                                     bias_row[0:1, ds(GS * h, GS)])
                if act is None:
                    nc.vector.tensor_copy(out_row[0:1, ds(GS * h, GS)], pre[:])
                else:
                    nc.scalar.activation(out_row[0:1, ds(GS * h, GS)],
                                         pre[:], act)

        def a_chain(vsb, vps, rowp, hw, hb, gw, gb, bw, bb, tagn, sc_b, of_b):
            def row_load(name):
                t = rowp.tile([1, D], f32, tag="arow_b", name=f"row_{name}",
                              bufs=2)
                nc.scalar.dma_start(t[:], ap[name][0:D])
                return t

            hb_row = row_load(hb)
            h_row = rowp.tile([1, D], f32, tag="h_row", name=f"h_{tagn}")
            vec_layer(vsb, vps, rowp, hw, zc_f, hb_row, AF.Silu, h_row)
            h_row_r = rowp.tile([1, D], f32r, tag="h_row_r", name=f"hr_{tagn}")
            nc.vector.tensor_copy(h_row_r[:], h_row[:])
            h_c = cols.tile([P, DC], f32r, tag=f"c_{tagn}", name=f"c_{tagn}")
            for j in range(DC):
                nc.sync.dma_start(h_c[:, j:j + 1], h_row_r[0:1, ds(P * j, P)])
            gb_row = row_load(gb)
            sc_row = rowp.tile([1, D], f32, tag="sc_row", name=f"sc_{tagn}")
            vec_layer(vsb, vps, rowp, gw, h_c, gb_row, None, sc_row)
            pe_bcast(vps, rowp, sc_row, sc_b, D)
            bb_row = row_load(bb)
            of_row = rowp.tile([1, D], f32, tag="of_row", name=f"of_{tagn}")
            vec_layer(vsb, vps, rowp, bw, h_c, bb_row, None, of_row)
            pe_bcast(vps, rowp, of_row, of_b, D)

        # =========== phase A1: adaLN1 conditioning vectors (f32r) ===========
        with (
            tc.tile_pool(name="vec1_sb", bufs=1) as vsb1,
            tc.tile_pool(name="row1_sb", bufs=1) as rowp1,
            tc.tile_pool(name="vec1_ps", bufs=2, space="PSUM") as vps1,
        ):
            a_chain(vsb1, vps1, rowp1, "h1w", "h1b", "g1w", "g1b",
                    "be1w", "be1b", "h1", scale1_b, offset1_b)
        # manual pool stacks (LIFO per SBUF side)
        s_woh = contextlib.ExitStack()   # left (created first: popped last)
        s_qT = contextlib.ExitStack()    # left
        s_x1n = contextlib.ExitStack()   # left
        s_x2T = contextlib.ExitStack()   # left
        s_w1 = contextlib.ExitStack()    # left
        s_w2 = contextlib.ExitStack()    # left (E only)
        s_kv = contextlib.ExitStack()    # right (kpT/vpe, lingers under aoT)
        s_pef = contextlib.ExitStack()   # right (pefE/F + colsums, popped at KV end)
        s_aoT = contextlib.ExitStack()   # right
        s_hm = contextlib.ExitStack()    # right
        try:
            # ===== phase B: adaLN1 + transposes + qT =====
            qT_p = s_qT.enter_context(tc.tile_pool(name="qT", bufs=1))
            qT = [[qT_p.tile([P, GS], f16, tag=f"qT_{j}_{g}", name=f"qT_{j}_{g}")
                   for g in range(NG)] for j in range(DC)]
            x1n_p = s_x1n.enter_context(tc.tile_pool(name="x1nat", bufs=1))
            x1n = []
            with (
                tc.tile_pool(name="wq_sb", bufs=1) as wqsb,
                tc.tile_pool(name="ln1_sb", bufs=2) as ln_sb,
                tc.tile_pool(name="x1Trot", bufs=1) as x1t_p,
                tc.tile_pool(name="tp1_ps", bufs=2, space="PSUM") as ln_ps,
                tc.tile_pool(name="q_ps", bufs=3, space="PSUM") as qps,
            ):
                wq_r = []
                for j in range(DC):
                    wf = wqsb.tile([P, D], f32, tag="wq_f", name="wq_f",
                                   bufs=2)
                    nc.scalar.dma_start(wf[:], ap["wq"][ds(P * j, P), :])
                    wr = wqsb.tile([P, D], f16, tag=f"wq_r{j}", name=f"wq_r{j}")
                    nc.scalar.copy(wr[:], wf[:])
                    wq_r.append(wr)
                bq_c = col_load("bq", DC)
                bk_c = col_load("bk", DC)
                Fb_c = col_load("Fb", KC)
                for g in range(NG):
                    x1T_g = [x1t_p.tile([P, GS], f16, tag=f"x1T_{j}",
                                        name=f"x1T_{j}") for j in range(DC)]
                    for ii in range(4):
                        i = 4 * g + ii
                        xt = ln_sb.tile([P, D], f32, tag="ln_in",
                                        name=f"ln_in{i}", bufs=4)
                        nc.sync.dma_start(xt[:], ap["x"][ds(P * i, P), :])
                        st = ln_sb.tile([P, 2, 6], f32, tag="ln_st",
                                        name="ln_st")
                        nc.vector.bn_stats(st[:, 0, :], xt[:, 0:GS])
                        nc.vector.bn_stats(st[:, 1, :], xt[:, GS:D])
                        mv = ln_sb.tile([P, 2], f32, tag="ln_mv", name="ln_mv")
                        nc.vector.bn_aggr(mv[:], st[:])
                        sd = ln_sb.tile([P, 1], f32, tag="ln_sd",
                                        name="ln_sd")
                        nc.scalar.activation(sd[:], mv[:, 1:2], AF.Sqrt,
                                             bias=eps_t[:])
                        rstd = ln_sb.tile([P, 1], f32, tag="ln_rstd",
                                          name="ln_rstd")
                        nc.vector.reciprocal_approx_fast(rstd[:], sd[:])
                        nmr = ln_sb.tile([P, 1], f32, tag="ln_nmr",
                                         name="ln_nmr")
                        nc.vector.tensor_scalar(nmr[:], mv[:, 0:1], rstd[:],
                                                -1.0, OP.mult, OP.mult)
                        xn = ln_sb.tile([P, D], f32, tag="ln_xn", name="ln_xn")
                        nc.scalar.activation(xn[:], xt[:], AF.Identity,
                                             bias=nmr[:], scale=rstd[:])
                        nc.vector.tensor_mul(xn[:], xn[:], scale1_b[:])
                        x1t = x1n_p.tile([P, D], f16, tag=f"nat{i}",
                                         name=f"nat{i}")
                        nc.vector.tensor_add(x1t[:], xn[:], offset1_b[:])
                        x1n.append(x1t)
                        for j in range(DC):
                            pt = ln_ps.tile([P, P], f16, tag="tp_ps",
                                            name="tp_ps")
                            nc.tensor.transpose(pt[:], x1t[:, ds(P * j, P)],
                                                ident_h[:])
                            nc.scalar.copy(
                                x1T_g[j][:, ds(P * ii, P)], pt[:])
                    for jo in range(DC):
                        pt = qps.tile([P, GS], f32, tag="q_ps", name="q_ps")
                        for j in range(DC):
                            nc.tensor.matmul(pt[:],
                                             wq_r[j][:, ds(P * jo, P)],
                                             x1T_g[j][:],
                                             start=(j == 0),
                                             stop=(j == DC - 1))
                        nc.scalar.activation(qT[jo][g][:], pt[:], AF.Identity,
                                             bias=bq_c[:, jo:jo + 1])

            # ===== phase B2: P_EF = x1^T @ [Ew|Fw], colsums fused after =====
            kv_sb = s_kv.enter_context(
                tc.tile_pool(name="kv_sb", bufs=1, side="right"))
            pef_sb = s_pef.enter_context(
                tc.tile_pool(name="pef_sb", bufs=1, side="right"))
            pefE = [pef_sb.tile([P, K], f16, tag=f"pefE{j}", name=f"pefE{j}")
                    for j in range(DC)]
            pefF = [pef_sb.tile([P, K], f16, tag=f"pefF{j}", name=f"pefF{j}")
                    for j in range(DC)]
            cs_row = pef_sb.tile([1, 2 * K], f32, tag="cs", name="cs")
            with tc.tile_pool(name="ef_sb", bufs=1) as efsb:
                ef_h = []
                for i in range(SC):
                    ff = efsb.tile([P, 2 * K], f32, tag="ef_f", name="ef_f",
                                   bufs=3)
                    nc.scalar.dma_start(ff[:, 0:K], ap["Ew"][ds(P * i, P), :])
                    nc.scalar.dma_start(ff[:, K:2 * K],
                                        ap["Fw"][ds(P * i, P), :])
                    t = efsb.tile([P, 2 * K], f16, tag=f"ef{i}", name=f"ef{i}")
                    nc.scalar.copy(t[:], ff[:])
                    ef_h.append(t)
                m1b_c = col_load("m1b", MC)
                with tc.tile_pool(name="pef_ps", bufs=1, space="PSUM") as pfps:
                    pef_ps = [pfps.tile([P, 2 * K], f32, tag=f"pefp{j}",
                                        name=f"pefp{j}") for j in range(DC)]
                    for i in range(SC):
                        for j in range(DC):
                            nc.tensor.matmul(pef_ps[j][:],
                                             x1n[i][:, ds(P * j, P)],
                                             ef_h[i][:],
                                             start=(i == 0), stop=(i == SC - 1))
                    for j in range(DC):
                        nc.scalar.copy(pefE[j][:], pef_ps[j][:, 0:K])
                        nc.scalar.copy(pefF[j][:], pef_ps[j][:, K:2 * K])
                with tc.tile_pool(name="cs_ps", bufs=1, space="PSUM") as csps:
                    cs_ps = csps.tile([1, 2 * K], f32, tag="cs_ps",
                                      name="cs_ps")
                    for i in range(SC):
                        nc.tensor.matmul(cs_ps[:], onescol_h[:], ef_h[i][:],
                                         start=(i == 0), stop=(i == SC - 1))
                    nc.vector.tensor_copy(cs_row[:], cs_ps[:])
            s_x1n.close()

            # ===== phase KV: k_projT, v_proj_ext =====
            kpT = [kv_sb.tile([P, K], f16, tag=f"kpT{j}", name=f"kpT{j}")
                   for j in range(DC)]
            vpe = [kv_sb.tile([P, 65 * H], f16, tag=f"vpe{c}", name=f"vpe{c}")
                   for c in range(KC)]
            with (
                tc.tile_pool(name="kv_w", bufs=1) as kvw,
                tc.tile_pool(name="kv_bias", bufs=1) as kvb,
                tc.tile_pool(name="kv_tmp", bufs=2) as kvt,
                tc.tile_pool(name="kv_ps", bufs=2, space="PSUM") as kvps,
            ):
                wk_r, wv_r = [], []
                for j in range(DC):
                    for nm, lst, tg in (("wk", wk_r, "wk"), ("wv", wv_r, "wv")):
                        wf = kvw.tile([P, D], f32, tag=f"{tg}_f",
                                      name=f"{tg}_f", bufs=2)
                        nc.sync.dma_start(wf[:], ap[nm][ds(P * j, P), :])
                        wr = kvw.tile([P, D], f16, tag=f"{tg}_r{j}",
                                      name=f"{tg}_r{j}")
                        nc.scalar.copy(wr[:], wf[:])
                        lst.append(wr)
                Eb_row = kvb.tile([1, K], f32, tag="Eb_row", name="Eb_row")
                nc.scalar.dma_start(Eb_row[:], ap["Eb"][0:K])
                Eb_b = kvb.tile([P, K], f32, tag="Eb_b", name="Eb_b")
                pe_bcast(kvps, kvt, Eb_row, Eb_b, K)
                csE_b = kvb.tile([P, K], f32, tag="csE_b", name="csE_b")
                pe_bcast(kvps, kvt, cs_row, csE_b, K)
                bv_row = kvb.tile([1, D], f32, tag="bv_row", name="bv_row")
                nc.scalar.dma_start(bv_row[:], ap["bv"][0:D])
                bv_b = kvb.tile([P, D], f32, tag="bv_b", name="bv_b")
                pe_bcast(kvps, kvt, bv_row, bv_b, D)
                csF_c = kvb.tile([P, KC], f32, tag="csF_c", name="csF_c")
                for c in range(KC):
                    nc.gpsimd.dma_start(csF_c[:, c:c + 1],
                                        cs_row[0:1, ds(K + P * c, P)])
                kp_bias = []
                for j in range(DC):
                    bt = kvb.tile([P, K], f32, tag=f"kpb{j}", name=f"kpb{j}")
                    nc.vector.tensor_scalar(bt[:], csE_b[:], bk_c[:, j:j + 1],
                                            None, OP.mult)
                    nc.vector.tensor_add(bt[:], bt[:], Eb_b[:])
                    kp_bias.append(bt)
                vp_bias = []
                for c in range(KC):
                    bt = kvb.tile([P, D], f32, tag=f"vpb{c}", name=f"vpb{c}")
                    nc.vector.tensor_scalar(bt[:], bv_b[:], csF_c[:, c:c + 1],
                                            Fb_c[:, c:c + 1], OP.mult, OP.add)
                    vp_bias.append(bt)

                for jo in range(DC):
                    pt = kvps.tile([P, K], f32, tag="kp_ps", name="kp_ps")
                    for j in range(DC):
                        nc.tensor.matmul(pt[:], wk_r[j][:, ds(P * jo, P)],
                                         pefE[j][:],
                                         start=(j == 0), stop=(j == DC - 1))
                    nc.vector.tensor_add(kpT[jo][:], pt[:], kp_bias[jo][:])
                for hf in range(2):
                    for c in range(KC):
                        pt = kvps.tile([P, GS], f32, tag="vp_ps", name="vp_ps")
                        for j in range(DC):
                            nc.tensor.matmul(pt[:], pefF[j][:, ds(P * c, P)],
                                             wv_r[j][:, ds(GS * hf, GS)],
                                             start=(j == 0), stop=(j == DC - 1))
                        tmp = kvt.tile([P, GS], f32, tag="vp_tmp",
                                       name="vp_tmp")
                        nc.vector.tensor_add(tmp[:], pt[:],
                                             vp_bias[c][:, ds(GS * hf, GS)])
                        for hh in range(8):
                            h = 8 * hf + hh
                            nc.vector.tensor_copy(vpe[c][:, ds(65 * h, 64)],
                                                  tmp[:, ds(64 * hh, 64)])
                for c in range(KC):
                    for h in range(H):
                        nc.vector.tensor_copy(vpe[c][:, ds(65 * h + 64, 1)],
                                              ones_f[:, 0:1])

            s_pef.close()

            # ===== load wo (f16) for C3 =====
            woh_sb = s_woh.enter_context(
                tc.tile_pool(name="woh_sb", bufs=1, side="right"))
            wo_r = []
            for j in range(DC):
                wf = woh_sb.tile([P, D], f32, tag="wo_f", name="wo_f", bufs=2)
                nc.sync.dma_start(wf[:], ap["wo"][ds(P * j, P), :])
                wr = woh_sb.tile([P, D], f16, tag=f"wo_r{j}", name=f"wo_r{j}")
                nc.vector.tensor_copy(wr[:], wf[:])
                wo_r.append(wr)

            # ===== phase C2: attention =====
            aoT_p = s_aoT.enter_context(
                tc.tile_pool(name="aoT", bufs=1, side="right"))
            aoT = [[aoT_p.tile([P, GS], f16, tag=f"aoT_{j}_{g}",
                               name=f"aoT_{j}_{g}")
                    for g in range(NG)] for j in range(DC)]
            with (
                tc.tile_pool(name="exp_sb", bufs=1) as expsb,
                tc.tile_pool(name="nrm_sb", bufs=1) as nrmsb,
                tc.tile_pool(name="sc_ps", bufs=4, space="PSUM") as scps,
                tc.tile_pool(name="av_ps", bufs=2, space="PSUM") as avps,
                tc.tile_pool(name="bc2_ps", bufs=2, space="PSUM") as bcps,
            ):
                def emit_scores_half(hg):
                    """scores+exp for the 8 heads of half-group hg (0..7),
                    interleaving base partitions 0/64 for PE row tiling."""
                    g, hb = hg // 2, (hg % 2) * 8
                    ets = {}
                    for ph in range(4):
                        for c in range(KC):
                            for e in range(2):
                                h = hb + 2 * ph + e
                                j, r0 = h // 2, 64 * (h % 2)
                                spt = scps.tile([P, GS], f32, tag="sc",
                                                name="sc")
                                nc.tensor.matmul(
                                    spt[:],
                                    kpT[j][r0:r0 + 64, ds(P * c, P)],
                                    qT[j][g][r0:r0 + 64, :],
                                    start=True, stop=True)
                                et = expsb.tile([P, GS], f16, tag="exp",
                                                name="exp", bufs=24)
                                nc.scalar.activation(et[:], spt[:], AF.Exp,
                                                     scale=0.125)
                                ets.setdefault(h, [None, None])[c] = et
                    return [(h, v) for h, v in ets.items()]

                def emit_av_half(exps, g):
                    items = []
                    for h, ets in exps:
                        apt = avps.tile([65, GS], f32, tag="av", name="av")
                        for c in range(KC):
                            nc.tensor.matmul(apt[:], vpe[c][:, ds(65 * h, 65)],
                                             ets[c][:],
                                             start=(c == 0), stop=(c == KC - 1))
                        den = nrmsb.tile([1, GS], f16, tag="den", name="den",
                                         bufs=26)
                        nc.vector.tensor_copy(den[:], apt[64:65, :])
                        to = nrmsb.tile([64, GS], f16, tag="to", name="to",
                                        bufs=26)
                        nc.vector.tensor_copy(to[:], apt[0:64, :])
                        items.append((h, den, to))
                    return items

                def emit_norm_half(items, g):
                    for h, den, to in items:
                        j, r0 = h // 2, 64 * (h % 2)
                        bpt = bcps.tile([64, GS], f32, tag="bc2", name="bc2")
                        nc.tensor.matmul(bpt[:], ones1_h[0:1, 0:64], den[:],
                                         start=True, stop=True)
                        rec = nrmsb.tile([64, GS], f32, tag="rec", name="rec",
                                         bufs=3)
                        nc.vector.reciprocal_approx_fast(rec[:], bpt[:])
                        nc.vector.tensor_mul(aoT[j][g][r0:r0 + 64, :],
                                             to[:], rec[:])

                sc_prev = None
                av_prev = None
                for hg in range(2 * NG):
                    cur = emit_scores_half(hg)
                    if sc_prev is not None:
                        items = emit_av_half(*sc_prev)
                        if av_prev is not None:
                            emit_norm_half(*av_prev)
                        av_prev = (items, sc_prev[1])
                    sc_prev = (cur, hg // 2)
                items = emit_av_half(*sc_prev)
                emit_norm_half(*av_prev)
                emit_norm_half(items, sc_prev[1])
            s_qT.close()

            # ===== phase A2: adaLN2 conditioning vectors =====
            with (
                tc.tile_pool(name="vec2_sb", bufs=1) as vsb2,
                tc.tile_pool(name="row2_sb", bufs=1) as rowp2,
                tc.tile_pool(name="vec2_ps", bufs=2, space="PSUM") as vps2,
            ):
                a_chain(vsb2, vps2, rowp2, "h2w", "h2b", "g2w", "g2b",
                        "be2w", "be2b", "h2", scale2_b, offset2_b)
                brow = rowp2.tile([1, D], f32, tag="brow", name="bo_row",
                                  bufs=2)
                nc.scalar.dma_start(brow[:], ap["bo"][0:D])
                pe_bcast(vps2, rowp2, brow, bo_b, D)
                brow2 = rowp2.tile([1, D], f32, tag="brow", name="m2b_row",
                                   bufs=2)
                nc.scalar.dma_start(brow2[:], ap["m2b"][0:D])
                pe_bcast(vps2, rowp2, brow2, m2b_b, D)

            # ===== phase C3: wo + residual + adaLN2 + transposes, fused =====
            x2T_p = s_x2T.enter_context(tc.tile_pool(name="x2T", bufs=1))
            x2T = [x2T_p.tile([P, S], f16, tag=f"x2T_{j}", name=f"x2T_{j}")
                   for j in range(DC)]
            with (
                tc.tile_pool(name="c3_sb", bufs=1) as c3sb,
                tc.tile_pool(name="wo_ps", bufs=3, space="PSUM") as wops,
                tc.tile_pool(name="tp2_ps", bufs=2, space="PSUM") as tp2ps,
            ):
                pend = []
                for i in range(SC):
                    g, c = i // 4, (i % 4) * P
                    xt = c3sb.tile([P, D], f32, tag="res_x", name="res_x",
                                   bufs=2)
                    nc.sync.dma_start(xt[:], ap["x"][ds(P * i, P), :])
                    at = c3sb.tile([P, D], f32, tag="attn_nat",
                                   name="attn_nat", bufs=3)
                    for hf in range(2):
                        pt = wops.tile([P, GS], f32, tag="wo_ps", name="wo_ps")
                        for j in range(DC):
                            nc.tensor.matmul(pt[:], aoT[j][g][:, ds(c, P)],
                                             wo_r[j][:, ds(GS * hf, GS)],
                                             start=(j == 0), stop=(j == DC - 1))
                        tm = c3sb.tile([P, GS], f32, tag="wo_tmp",
                                       name="wo_tmp", bufs=3)
                        nc.vector.tensor_add(tm[:], pt[:],
                                             bo_b[:, ds(GS * hf, GS)])
                        nc.vector.tensor_add(at[:, ds(GS * hf, GS)], tm[:],
                                             xt[:, ds(GS * hf, GS)])
                    nc.sync.dma_start(attn_sc[i][:], at[:])
                    st = c3sb.tile([P, 2, 6], f32, tag="ln2_st",
                                   name="ln2_st", bufs=2)
                    nc.vector.bn_stats(st[:, 0, :], at[:, 0:GS])
                    nc.vector.bn_stats(st[:, 1, :], at[:, GS:D])
                    mv = c3sb.tile([P, 2], f32, tag="ln2_mv", name="ln2_mv")
                    nc.vector.bn_aggr(mv[:], st[:])
                    sd = c3sb.tile([P, 1], f32, tag="ln2_sd",
                                   name="ln2_sd")
                    nc.scalar.activation(sd[:], mv[:, 1:2], AF.Sqrt,
                                         bias=eps_t[:])
                    rstd = c3sb.tile([P, 1], f32, tag="ln2_rstd",
                                     name="ln2_rstd")
                    nc.vector.reciprocal_approx_fast(rstd[:], sd[:])
                    nmr = c3sb.tile([P, 1], f32, tag="ln2_nmr", name="ln2_nmr")
                    nc.vector.tensor_scalar(nmr[:], mv[:, 0:1], rstd[:],
                                            -1.0, OP.mult, OP.mult)
                    xn = c3sb.tile([P, D], f32, tag="ln2_xn", name="ln2_xn")
                    nc.scalar.activation(xn[:], at[:], AF.Identity,
                                         bias=nmr[:], scale=rstd[:])
                    nc.vector.tensor_mul(xn[:], xn[:], scale2_b[:])
                    x2t = c3sb.tile([P, D], f16, tag="x2nat", name="x2nat",
                                    bufs=3)
                    nc.vector.tensor_add(x2t[:], xn[:], offset2_b[:])
                    pend.append((i, x2t))
                    if len(pend) > 1:
                        pi, px = pend.pop(0)
                        for j in range(DC):
                            pt = tp2ps.tile([P, P], f16, tag="tp2_ps",
                                            name="tp2_ps")
                            nc.tensor.transpose(pt[:], px[:, ds(P * j, P)],
                                                ident_h[:])
                            nc.scalar.copy(x2T[j][:, ds(P * pi, P)], pt[:])
                for pi, px in pend:
                    for j in range(DC):
                        pt = tp2ps.tile([P, P], f16, tag="tp2_ps",
                                        name="tp2_ps")
                        nc.tensor.transpose(pt[:], px[:, ds(P * j, P)],
                                            ident_h[:])
                        nc.scalar.copy(x2T[j][:, ds(P * pi, P)], pt[:])
            s_aoT.close()
            s_woh.close()
            s_kv.close()

            # ===== phase E: MLP per token group =====
            hm_p = s_hm.enter_context(
                tc.tile_pool(name="hm", bufs=1, side="right"))
            hm = [hm_p.tile([P, GS], f16, tag=f"hm{m}", name=f"hm{m}")
                  for m in range(MC)]
            # w2 resident f16, filled during m1(g0) (f32 stream + DVE cast)
            w2sb_p = s_w2.enter_context(tc.tile_pool(name="w2sb", bufs=1))
            w2sb = [None] * MC

            def w2_fill(ms):
                for m in ms:
                    wf = w2sb_p.tile([P, D], f32, tag="w2_f", name="w2_f",
                                     bufs=2)
                    nc.scalar.dma_start(wf[:], ap["m2w"][ds(P * m, P), :])
                    t = w2sb_p.tile([P, D], f16, tag=f"w2_{m}", name=f"w2_{m}")
                    nc.vector.tensor_copy(t[:], wf[:])
                    w2sb[m] = t
            # w1 streamed f32 in quarter-tiles per (group, j) + scalar cast;
            # m1 runs in m-eighth passes; psum: 4 shared tags x 2 banks for
            # both m1 ([P,512] slices) and the 1024-wide m2 accumulation.
            QW = MLP // 4  # 1024 m-columns per quarter load
            with (
                tc.tile_pool(name="w1st", bufs=1) as w1st,
                tc.tile_pool(name="e_sb", bufs=1) as esb,
                tc.tile_pool(name="mlp_ps", bufs=1, space="PSUM") as mlpps,
            ):
                for g in range(NG):
                    for q in range(4):
                        wts = []
                        for j in range(DC):
                            wf = w1st.tile([P, QW], f32, tag="w1_f",
                                           name="w1_f", bufs=3)
                            nc.sync.dma_start(
                                wf[:], ap["m1w"][ds(P * j, P), ds(QW * q, QW)])
                            wt = w1st.tile([P, QW], f16, tag=f"w1_h{j}",
                                           name=f"w1_h{j}", bufs=1)
                            nc.scalar.copy(wt[:], wf[:])
                            wts.append(wt)
                        for e8 in range(2):
                            pts = [mlpps.tile([P, GS], f32, tag=f"p{mm}",
                                              name=f"p{mm}",
                                              padded_shape=[P, 2 * GS])
                                   for mm in range(4)]
                            for j in range(DC):
                                for mm in range(4):
                                    nc.tensor.matmul(
                                        pts[mm][:],
                                        wts[j][:, ds(GS * e8 + P * mm, P)],
                                        x2T[j][:, ds(GS * g, GS)],
                                        start=(j == 0), stop=(j == DC - 1))
                            for mm in range(4):
                                m = 8 * q + 4 * e8 + mm
                                nc.scalar.activation(hm[m][:], pts[mm][:],
                                                     AF.Gelu,
                                                     bias=m1b_c[:, m:m + 1])
                        if g == 0:
                            w2_fill(range(8 * q, 8 * q + 8))
                    for half in range(2):
                        m2p = [mlpps.tile([P, GS], f32, tag=f"p{ss}",
                                          name=f"m2p{ss}",
                                          padded_shape=[P, 2 * GS])
                               for ss in range(4)]
                        for m in range(MC):
                            for ss in range(4):
                                nc.tensor.matmul(
                                    m2p[ss][:], hm[m][:, ds(P * ss, P)],
                                    w2sb[m][:, ds(GS * half, GS)],
                                    start=(m == 0), stop=(m == MC - 1))
                        for ss in range(4):
                            i = 4 * g + ss
                            rt = esb.tile([P, GS], f32, tag="res_a",
                                          name="res_a", bufs=2)
                            nc.sync.dma_start(
                                rt[:], attn_sc[i][:, ds(GS * half, GS)])
                            tm = esb.tile([P, GS], f32, tag="e_tmp",
                                          name="e_tmp", bufs=2)
                            nc.vector.tensor_add(tm[:], m2p[ss][:],
                                                 m2b_b[:, ds(GS * half, GS)])
                            ot = esb.tile([P, GS], f32, tag="e_out",
                                          name="e_out", bufs=2)
                            nc.vector.tensor_add(ot[:], tm[:], rt[:])
                            nc.sync.dma_start(
                                out[ds(P * i, P), ds(GS * half, GS)], ot[:])
        finally:
            for s in (s_w2, s_w1, s_x2T, s_qT, s_x1n, s_woh, s_hm, s_aoT,
                      s_pef, s_kv):
                s.close()


def kernel(**inputs):
    nc = build()
    x = np.ascontiguousarray(inputs["x"], dtype=np.float32)
    z = np.ascontiguousarray(inputs["z"], dtype=np.float32)
    base = {}
    for nm, _, _ in W2D + W2DR:
        base[nm] = np.ascontiguousarray(inputs[nm], dtype=np.float32)
    for nm, _ in W1D:
        base[nm] = np.ascontiguousarray(inputs[nm], dtype=np.float32)
    in_maps = []
    for c in range(B):
        m = dict(base)
        m["x"] = x[c]
        m["z"] = z[c:c + 1]
        in_maps.append(m)
    res = run_bass_kernel_spmd(nc, in_maps, list(range(B)))
    _cache["last"] = res
    return np.stack([res.results[c]["out"] for c in range(B)], axis=0)
